# revision 12
# baseline (speedup 1.0000x reference)
"""NequIP-style GNN message passing layer on 8 Trainium2 NeuronCores.

Strategy (edges partitioned across cores per the sharding hint):
  * Host: sort edges by destination, split into 8 contiguous shards, gather
    source-node features per edge, precompute small per-edge feature products
    (sy0, C, v*y0, y1-replicated), pad edges into fixed-size "groups" whose
    destinations span at most a 256-node window, and build per-group one-hot
    scatter matrices.  All tensor-product path constants and the 1/sqrt(deg)
    normalization are folded into a column-permuted copy of W2 (j-major
    within each 16x16 block so the i-contraction is a contiguous
    16-partition column sum).
  * Device (per 512-edge tile, features-on-partitions / edges-on-free):
      - MLP1/MLP2 as fp32r matmuls -> per-edge TP weights wT [1024, T]
      - PE "replication" matmuls broadcast per-edge features across
        partition blocks; DVE bf16 elementwise products; PE "column-sum"
        matmuls with constant 0/1 selection matrices contract over i
      - edge features are PE-transposed back to edges-on-partitions and
        scatter-summed into a 256-node PSUM window via one-hot bf16 matmuls
  * Host: add the per-window partial sums from all cores into [10000, 64].
"""

import sys

if "/opt/trn_rl_repo" not in sys.path:
    sys.path.insert(0, "/opt/trn_rl_repo")

import numpy as np
import ml_dtypes

import concourse.bass as bass
import concourse.mybir as mybir
import concourse.tile as tile
from concourse.tile import ScopedClock
from concourse.bass_utils import run_bass_kernel_spmd

# ---------------- problem constants (hardcoded per contract) ----------------
N = 10000
E = 160000
MUL = 16
DIM_IN = 64
DIM_SH = 4
EMB = 18
HID = 256
WNUM = 1024

NCORES = 8
T = 512          # edges per tile
SUBT = 128       # edges per subtile (partition dim for scatter)
WIN = 256        # node window per group (2 PSUM-bank halves of 128)
GT = 4           # tiles per group
GROUP_E = GT * T  # 2048 edges per group (avg edges per 256 nodes = 4096 -> groups
                  # are usually edge-capped, minimizing padding)

BF16 = ml_dtypes.bfloat16
F32 = np.float32

_BUILD_CACHE = {}
_last_exec_ns = None
_last_results = None


# --------------------------------------------------------------------------
# Tile tail-drain fix: walrus's CTRL lowering in this container accepts only
# one sem wait per Drain; TileContext's end-of-kernel drain can carry several.
# Split them across consecutive drains (same semantics: waits AND together).
# --------------------------------------------------------------------------
def _patched_drain_and_barrier(self, tick_clock, wait_clock):
    drain_inst = self.nc.sync.drain()
    wait_clock.add_sem_waits(
        drain_inst.ins, ScopedClock({None: tick_clock.global_clock})
    )
    si = drain_inst.ins.sync_info
    if si is not None and si.on_wait and len(si.on_wait) > 1:
        waits = list(si.on_wait)
        drain_inst.ins.sync_info = mybir.SyncInfo(
            on_wait=waits[:1], on_update=list(si.on_update or [])
        )
        for w in waits[1:]:
            extra = self.nc.sync.drain()
            extra.ins.sync_info = mybir.SyncInfo(on_wait=[w], on_update=[])
    self.nc.all_engine_barrier()
    assert self.sems is not None
    popped = self.nc._tile_sem_poison_stack.pop()
    assert popped is self._sem_poison
    self.nc.clear_and_free_semaphores(list(self.sems.allocated().values()))
    self.nc.all_engine_barrier()


if getattr(tile.TileContext, "_drain_patch", None) is not True:
    tile.TileContext._drain_and_barrier = _patched_drain_and_barrier
    tile.TileContext._drain_patch = True


def _split_excess_waits(nc: bass.Bass, maxw: int = 1) -> None:
    """walrus's setupSyncWait in this container rejects instructions with
    more than one sem wait.  Move excess waits onto same-engine NOPs placed
    immediately before the instruction (engine stalls there first; identical
    semantics since waits AND together)."""
    for f in nc.m.functions:
        for bb in f.blocks:
            rewritten = []
            changed = False
            for inst in bb.instructions:
                si = inst.sync_info
                if si is not None and si.on_wait and len(si.on_wait) > maxw:
                    waits = list(si.on_wait)
                    extra, keep = waits[:-maxw], waits[-maxw:]
                    for i, w in enumerate(extra):
                        nop = mybir.InstNoOp(
                            name=f"{inst.name}-ws{i}",
                            engine=inst.engine,
                            ins=[],
                            outs=[],
                        )
                        nop.sync_info = mybir.SyncInfo(on_wait=[w], on_update=[])
                        rewritten.append(nop)
                    inst.sync_info = mybir.SyncInfo(
                        on_wait=keep, on_update=list(si.on_update or [])
                    )
                    changed = True
                rewritten.append(inst)
            if changed:
                bb.instructions = rewritten


# ---------------------------- device program ------------------------------
def _build(ng: int, split_waits: bool = True, act: str = 'silu') -> bass.Bass:
    """Build the per-core Bass program for ng groups (= ng*GT tiles)."""
    f32 = mybir.dt.float32
    f32r = mybir.dt.float32r
    bf16 = mybir.dt.bfloat16

    nt = ng * GT
    epc = nt * T
    nsub = epc // SUBT

    nc = bass.Bass(trn_type="TRN2")

    embT = nc.dram_tensor("embt", [EMB, epc], f32r, kind="ExternalInput")
    featA = nc.dram_tensor("feata", [16, 6, epc], bf16, kind="ExternalInput")
    featB = nc.dram_tensor("featb", [48, epc], bf16, kind="ExternalInput")
    oh = nc.dram_tensor("oh", [nsub, SUBT, WIN], bf16, kind="ExternalInput")
    w1 = nc.dram_tensor("w1", [EMB, HID], f32r, kind="ExternalInput")
    b1 = nc.dram_tensor("b1", [HID, 1], f32, kind="ExternalInput")
    w2p = nc.dram_tensor("w2p", [HID, WNUM], f32r, kind="ExternalInput")
    s16t = nc.dram_tensor("s16t", [16, 128], bf16, kind="ExternalInput")
    selab = nc.dram_tensor("selab", [128, 32], bf16, kind="ExternalInput")
    selv = nc.dram_tensor("selv", [128, 6, 48], bf16, kind="ExternalInput")
    s48 = nc.dram_tensor("s48", [16, 48], bf16, kind="ExternalInput")
    ident = nc.dram_tensor("ident", [64, 64], bf16, kind="ExternalInput")
    out = nc.dram_tensor("out", [ng * WIN, DIM_IN], f32, kind="ExternalOutput")

    with tile.TileContext(nc) as tc:
        with (
            tc.tile_pool(name="consts", bufs=1) as cpool,
            tc.tile_pool(name="io", bufs=3) as iop,
            tc.tile_pool(name="hbuf", bufs=2) as hpool,
            tc.tile_pool(name="wbuf", bufs=3) as wpool,
            tc.tile_pool(name="repbuf", bufs=3) as repsb,
            tc.tile_pool(name="prod", bufs=2) as prodp,
            tc.tile_pool(name="efp", bufs=2) as efp,
            tc.tile_pool(name="winp", bufs=2) as winp,
            tc.tile_pool(name="psw", bufs=2, space="PSUM") as psw,
            tc.tile_pool(name="psrep", bufs=2, space="PSUM") as psrep,
            tc.tile_pool(name="pssmall", bufs=1, space="PSUM") as pssmall,
            tc.tile_pool(name="psseg", bufs=1, space="PSUM") as psseg,
            tc.tile_pool(name="pstr", bufs=1, space="PSUM") as pstr,
        ):
            # ---- constants, loaded once ----
            w1_sb = cpool.tile([EMB, HID], f32r)
            nc.sync.dma_start(out=w1_sb[:], in_=w1[:])
            b1_sb = cpool.tile([128, 2], f32)  # col k = b1[k*128:(k+1)*128]
            nc.sync.dma_start(
                out=b1_sb[:], in_=b1.rearrange("(c p) o -> p (c o)", p=128)
            )
            w2p_sb = cpool.tile([128, 2, WNUM], f32r)
            nc.sync.dma_start(
                out=w2p_sb[:], in_=w2p.rearrange("(c p) m -> p c m", p=128)
            )
            s16t_sb = cpool.tile([16, 128], bf16)
            nc.sync.dma_start(out=s16t_sb[:], in_=s16t[:])
            selab_sb = cpool.tile([128, 32], bf16)
            nc.sync.dma_start(out=selab_sb[:], in_=selab[:])
            selv_sb = cpool.tile([128, 6, 48], bf16)
            nc.sync.dma_start(out=selv_sb[:], in_=selv[:])
            s48_sb = cpool.tile([16, 48], bf16)
            nc.sync.dma_start(out=s48_sb[:], in_=s48[:])
            ident_sb = cpool.tile([64, 64], bf16)
            nc.sync.dma_start(out=ident_sb[:], in_=ident[:])

            for g in range(ng):
                win_a = winp.tile([128, 64], f32, tag="wina")
                win_b = winp.tile([128, 64], f32, tag="winb")
                for tt in range(GT):
                    t = g * GT + tt
                    # ---------------- loads ----------------
                    emb_t = iop.tile([EMB, T], f32r, tag="embt")
                    nc.sync.dma_start(
                        out=emb_t[:], in_=embT[:, t * T : (t + 1) * T]
                    )
                    fa_t = iop.tile([16, 6, T], bf16, tag="feata")
                    nc.sync.dma_start(
                        out=fa_t[:], in_=featA[:, :, t * T : (t + 1) * T]
                    )
                    fb_t = iop.tile([48, T], bf16, tag="featb")
                    nc.sync.dma_start(
                        out=fb_t[:], in_=featB[:, t * T : (t + 1) * T]
                    )
                    oh_t = iop.tile([SUBT, 4, WIN], bf16, tag="oh")
                    nc.sync.dma_start(
                        out=oh_t[:],
                        in_=oh[4 * t : 4 * t + 4].rearrange("s p c -> p s c"),
                    )

                    # ---------------- MLP1: hT = silu(W1.T @ embT + b1) ----
                    h_sb = []
                    for k in range(2):
                        ps_h = psw.tile([128, T], f32, tag="psw")
                        nc.tensor.matmul(
                            ps_h[:],
                            w1_sb[:, k * 128 : (k + 1) * 128],
                            emb_t[:],
                            start=True,
                            stop=True,
                        )
                        h_k = hpool.tile([128, T], f32r, tag=f"h{k}")
                        nc.scalar.activation(
                            h_k[:],
                            ps_h[:],
                            mybir.ActivationFunctionType.Silu
                            if act == "silu"
                            else mybir.ActivationFunctionType.Sigmoid,
                            bias=b1_sb[:, k : k + 1],
                        )
                        h_sb.append(h_k)

                    # ---------------- MLP2: wT = W2P.T @ hT ----------------
                    w_sb = []
                    for m in range(8):
                        ps_w = psw.tile([128, T], f32, tag="psw")
                        for k in range(2):
                            nc.tensor.matmul(
                                ps_w[:],
                                w2p_sb[:, k, m * 128 : (m + 1) * 128],
                                h_sb[k][:],
                                start=(k == 0),
                                stop=(k == 1),
                            )
                        w_m = wpool.tile([128, T], bf16, tag=f"w{m}")
                        nc.scalar.copy(w_m[:], ps_w[:])
                        w_sb.append(w_m)

                    # -------- PE replication of per-edge modulators --------
                    # (each [128, T] PSUM block is the 16-row source repeated
                    #  8x; identical for both m-blocks of a path)
                    def replicate(src_rows, to_dve, tag):
                        ps_r = psrep.tile([128, T], f32, tag="psrep")
                        nc.tensor.matmul(
                            ps_r[:], s16t_sb[:], src_rows, start=True, stop=True
                        )
                        r_sb = repsb.tile([128, T], bf16, tag=tag)
                        if to_dve:
                            nc.vector.tensor_copy(r_sb[:], ps_r[:])
                        else:
                            nc.scalar.copy(r_sb[:], ps_r[:])
                        return r_sb

                    srep = replicate(fa_t[:, 0, :], False, "srep")
                    sy0rep = replicate(fa_t[:, 1, :], False, "sy0rep")
                    crep = replicate(fa_t[:, 2, :], False, "crep")
                    vy0rep = [
                        replicate(fa_t[:, 3 + a, :], True, f"vy0r{a}")
                        for a in range(3)
                    ]

                    # -------- DVE products (bf16, 2x mode) --------
                    def prod(w_m, rep, tag):
                        p = prodp.tile([128, T], bf16, tag=tag)
                        nc.vector.tensor_mul(p[:], w_m[:], rep[:])
                        return p

                    p0 = [prod(w_sb[b], sy0rep, f"p0{b}") for b in range(2)]
                    p1 = [prod(w_sb[2 + b], crep, f"p1{b}") for b in range(2)]
                    p2 = [prod(w_sb[4 + b], srep, f"p2{b}") for b in range(2)]
                    p3 = [
                        [
                            prod(w_sb[6 + b], vy0rep[a], f"p3{a}{b}")
                            for b in range(2)
                        ]
                        for a in range(3)
                    ]

                    # -------- PE column sums (contract over i) --------
                    # (PE tile_position: 16-row outputs need base partition in
                    #  {0,32,64,96}; 48-row outputs need {0,64})
                    small_ps = pssmall.tile([128, T], f32, tag="small")
                    small2_ps = pssmall.tile([64, T], f32, tag="small2")
                    os_ps = small_ps[0:16, :]
                    d_ps = small_ps[32:48, :]
                    drep_ps = small_ps[64:112, :]
                    ov_ps = small2_ps[0:48, :]

                    sel_a = selab_sb[:, 0:16]
                    sel_b = selab_sb[:, 16:32]
                    nc.tensor.matmul(os_ps, sel_a, p0[0][:], start=True, stop=False)
                    nc.tensor.matmul(os_ps, sel_b, p0[1][:], start=False, stop=False)
                    nc.tensor.matmul(os_ps, sel_a, p1[0][:], start=False, stop=False)
                    nc.tensor.matmul(os_ps, sel_b, p1[1][:], start=False, stop=True)

                    nc.tensor.matmul(d_ps, sel_a, p2[0][:], start=True, stop=False)
                    nc.tensor.matmul(d_ps, sel_b, p2[1][:], start=False, stop=True)
                    d_sb = repsb.tile([16, T], bf16, tag="dsb")
                    nc.scalar.copy(d_sb[:], d_ps)

                    nc.tensor.matmul(drep_ps, s48_sb[:], d_sb[:], start=True, stop=True)
                    drep_sb = repsb.tile([48, T], bf16, tag="drepsb")
                    nc.scalar.copy(drep_sb[:], drep_ps)

                    first = True
                    for b in range(2):
                        for a in range(3):
                            nc.tensor.matmul(
                                ov_ps,
                                selv_sb[:, b * 3 + a, :],
                                p3[a][b][:],
                                start=first,
                                stop=(b == 1 and a == 2),
                            )
                            first = False
                    ov_sb = repsb.tile([48, T], bf16, tag="ovsb")
                    nc.scalar.copy(ov_sb[:], ov_ps)

                    # -------- edge features (16-row s part, 48-row v part) ----
                    ef_s = efp.tile([16, T], bf16, tag="efs")
                    nc.scalar.copy(ef_s[:], os_ps)
                    ef_v = efp.tile([48, T], bf16, tag="efv")
                    tmpv = efp.tile([48, T], bf16, tag="tmpv")
                    nc.vector.tensor_mul(tmpv[:], drep_sb[:], fb_t[:])
                    nc.vector.tensor_add(ef_v[:], tmpv[:], ov_sb[:])

                    # -------- scatter: transpose + one-hot matmul --------
                    # (A/B window halves must be *sequential* accumulation
                    #  groups: concurrent start/stop groups in one PSUM bank
                    #  are illegal)
                    seg_ps = psseg.tile([128, 128], f32, tag="seg")
                    ef_rs = []
                    for sub in range(4):
                        tr_ps = pstr.tile([128, 64], bf16, tag="tr")
                        esl = slice(sub * 128, (sub + 1) * 128)
                        nc.tensor.transpose(
                            tr_ps[:, 0:16], ef_s[:, esl], ident_sb[0:16, 0:16]
                        )
                        nc.tensor.transpose(
                            tr_ps[:, 16:64], ef_v[:, esl], ident_sb[0:48, 0:48]
                        )
                        ef_r = efp.tile([128, 64], bf16, tag=f"efr{sub}")
                        nc.scalar.copy(ef_r[:], tr_ps[:])
                        ef_rs.append(ef_r)
                    for sub in range(4):
                        nc.tensor.matmul(
                            seg_ps[:, 0:64],
                            oh_t[:, sub, 0:128],
                            ef_rs[sub][:],
                            start=(sub == 0),
                            stop=(sub == 3),
                        )
                    for sub in range(4):
                        nc.tensor.matmul(
                            seg_ps[:, 64:128],
                            oh_t[:, sub, 128:256],
                            ef_rs[sub][:],
                            start=(sub == 0),
                            stop=(sub == 3),
                        )

                    if tt == 0:
                        nc.scalar.copy(win_a[:], seg_ps[:, 0:64])
                        nc.scalar.copy(win_b[:], seg_ps[:, 64:128])
                    else:
                        nc.vector.tensor_add(win_a[:], win_a[:], seg_ps[:, 0:64])
                        nc.vector.tensor_add(win_b[:], win_b[:], seg_ps[:, 64:128])

                nc.sync.dma_start(
                    out=out[g * WIN : g * WIN + 128, :], in_=win_a[:]
                )
                nc.sync.dma_start(
                    out=out[g * WIN + 128 : (g + 1) * WIN, :], in_=win_b[:]
                )

    if split_waits:
        _split_excess_waits(nc)
    return nc


# ---------------------------- host-side prep ------------------------------
def _plan_groups(dst: np.ndarray):
    """Greedy cut into groups of <= GROUP_E edges spanning < WIN node ids."""
    groups = []
    i, n = 0, len(dst)
    while i < n:
        lo = int(dst[i])
        j = min(i + GROUP_E, int(np.searchsorted(dst, lo + WIN, side="left")))
        j = max(j, i + 1)
        groups.append((i, j, lo))
        i = j
    return groups


def _sel_constants():
    r = np.arange(128)
    s16t = (r[None, :] % 16 == np.arange(16)[:, None]).astype(BF16)
    selab = np.zeros((128, 32), BF16)
    selab[r, r // 16] = 1
    selab[r, 16 + r // 16 + 8] = 1
    selv = np.zeros((128, 6, 48), BF16)
    for b in range(2):
        for a in range(3):
            selv[r, b * 3 + a, (r // 16 + 8 * b) * 3 + a] = 1
    s48 = (np.arange(48)[None, :] // 3 == np.arange(16)[:, None]).astype(BF16)
    ident = np.eye(64, dtype=BF16)
    return s16t, selab, selv, s48, ident


def _permuted_w2(W2: np.ndarray) -> np.ndarray:
    inv3 = 1.0 / np.sqrt(3.0)
    c_s = np.sqrt(1.0 / (2 * MUL))
    c_v = np.sqrt(3.0 / (2 * MUL))
    z = 1.0 / np.sqrt(E / float(N))
    scale = np.array([c_s * z, c_s * inv3 * z, c_v * inv3 * z, c_v * inv3 * z])
    c = np.arange(WNUM)
    p, rem = c // 256, c % 256
    jj, ii = rem // 16, rem % 16
    src = p * 256 + ii * 16 + jj
    return (W2[:, src] * scale[p][None, :]).astype(F32)


def kernel(
    node_features,
    edge_attr,
    edge_embedding,
    W1,
    b1,
    W2,
    b2,
    edge_src,
    edge_dst,
):
    node_features = np.asarray(node_features, F32)
    edge_attr = np.asarray(edge_attr, F32)
    edge_embedding = np.asarray(edge_embedding, F32)
    W1 = np.asarray(W1, F32)
    b1 = np.asarray(b1, F32)
    W2 = np.asarray(W2, F32)
    b2 = np.asarray(b2, F32)
    edge_src = np.asarray(edge_src)
    edge_dst = np.asarray(edge_dst)

    order = np.argsort(edge_dst, kind="stable")
    per_core = E // NCORES

    core_groups = []
    for c in range(NCORES):
        o = order[c * per_core : (c + 1) * per_core]
        core_groups.append((o, _plan_groups(edge_dst[o])))
    ng = max(len(g) for _, g in core_groups)

    nc = _BUILD_CACHE.get(ng)
    if nc is None:
        nc = _build(ng)
        _BUILD_CACHE[ng] = nc

    nt = ng * GT
    epc = nt * T
    nsub = epc // SUBT
    w2p = _permuted_w2(W2)
    s16t, selab, selv, s48, ident = _sel_constants()

    in_maps = []
    los_all = []
    for c in range(NCORES):
        o, groups = core_groups[c]
        perm = np.full(epc, -1, np.int64)
        los = np.zeros(ng, np.int64)
        col = np.zeros(epc, np.int64)
        for g, (i, j, lo) in enumerate(groups):
            k = j - i
            perm[g * GROUP_E : g * GROUP_E + k] = o[i:j]
            los[g] = lo
            col[g * GROUP_E : g * GROUP_E + k] = edge_dst[o[i:j]] - lo
        valid = perm >= 0
        pidx = np.where(valid, perm, 0)

        emb_p = np.where(
            valid[:, None], edge_embedding[pidx], 0.0
        ).astype(F32)
        x = node_features[edge_src[pidx]]
        y = edge_attr[pidx]
        s = x[:, :MUL]
        v3 = x[:, MUL:].reshape(-1, MUL, 3)
        y0 = y[:, 0:1]
        y1 = y[:, 1:4]
        sy0 = s * y0
        cfeat = (v3 * y1[:, None, :]).sum(-1)
        vy0a = (v3 * y0[:, :, None]).transpose(0, 2, 1).reshape(-1, 48)
        feat_a = np.concatenate([s, sy0, cfeat, vy0a], axis=1)
        feat_a[~valid] = 0.0
        feat_a = feat_a.T.reshape(6, 16, epc).transpose(1, 0, 2)
        feat_b = np.tile(y1, (1, MUL))
        feat_b[~valid] = 0.0

        ohm = np.zeros((epc, WIN), BF16)
        ohm[np.where(valid)[0], col[valid]] = 1

        in_maps.append(
            dict(
                embt=np.ascontiguousarray(emb_p.T),
                feata=np.ascontiguousarray(feat_a.astype(BF16)),
                featb=np.ascontiguousarray(feat_b.T.astype(BF16)),
                oh=ohm.reshape(nsub, SUBT, WIN),
                w1=W1,
                b1=b1.reshape(HID, 1),
                w2p=w2p,
                s16t=s16t,
                selab=selab,
                selv=selv,
                s48=s48,
                ident=ident,
            )
        )
        los_all.append(los)

    res = run_bass_kernel_spmd(nc, in_maps, core_ids=list(range(NCORES)))
    global _last_exec_ns, _last_results
    _last_exec_ns = res.exec_time_ns
    _last_results = res

    out = np.zeros((N, DIM_IN), F32)
    for c in range(NCORES):
        blocks = res.results[c]["out"]
        los = los_all[c]
        ngc = len(core_groups[c][1])
        for g in range(ngc):
            lo = int(los[g])
            hi = min(lo + WIN, N)
            out[lo:hi] += blocks[g * WIN : g * WIN + (hi - lo)]

    # b2 is zeros in this problem; if not, its (h-independent) contribution
    # to the tensor product is linear in the precomputed edge features and is
    # added exactly on the host.
    if np.any(b2):
        out += _b2_contribution(node_features, edge_attr, b2, edge_src, edge_dst)
    return out


def _b2_contribution(node_features, edge_attr, b2, edge_src, edge_dst):
    inv3 = 1.0 / np.sqrt(3.0)
    c_s = np.sqrt(1.0 / (2 * MUL))
    c_v = np.sqrt(3.0 / (2 * MUL))
    z = 1.0 / np.sqrt(E / float(N))
    w = b2.reshape(4, MUL, MUL)
    x = node_features[edge_src]
    s = x[:, :MUL]
    v = x[:, MUL:].reshape(-1, MUL, 3)
    y0 = edge_attr[:, 0]
    y1 = edge_attr[:, 1:4]
    out_s = c_s * (
        np.einsum("ij,e,ei->ej", w[0], y0, s)
        + inv3 * np.einsum("ij,ea,eia->ej", w[1], y1, v)
    )
    out_v = (c_v * inv3) * (
        np.einsum("ij,ei,ea->eja", w[2], s, y1)
        + np.einsum("ij,eia,e->eja", w[3], v, y0)
    )
    ef = np.concatenate([out_s, out_v.reshape(-1, 3 * MUL)], axis=1)
    out = np.zeros((N, DIM_IN), F32)
    np.add.at(out, edge_dst, ef)
    return (out * z).astype(F32)


# revision 13
# speedup vs baseline: 1.4304x; 1.4304x over previous
"""NequIP-style GNN message passing layer on 8 Trainium2 NeuronCores.

Strategy (edges partitioned across cores per the sharding hint):
  * Host: sort edges by destination, split into 8 contiguous shards, gather
    source-node features per edge, precompute the per-edge tensor-product
    modulators already replicated across the 128-partition layout the device
    needs (sy0, C, s, v*y0 per spherical component), pad edges into
    fixed-size groups whose destinations span < 256 node ids, and build
    per-group one-hot scatter matrices.  All TP path constants and the
    1/sqrt(deg) normalization are folded into a column-permuted copy of W2
    (j-major inside each 16x16 block so the i-contraction is a contiguous
    16-partition column sum).
  * Device (per 512-edge tile; features on partitions, edges on free dim):
      - MLP1/MLP2 as fp32r matmuls -> per-edge TP weights wT [1024, T] in
        paired PSUM banks [128, 2, T]
      - DVE elementwise products w * modulator, mostly straight out of PSUM
      - PE column-sum matmuls with constant 0/1 selection matrices contract
        over the multiplicity index i
      - edge features are PE-transposed to edges-on-partitions and
        scatter-summed via one-hot bf16 matmuls into a [64, 256] window
        accumulated in SBUF, DMA'd out per group (host transposes windows)
  * Host: add the per-window partial sums from all cores into [10000, 64].
"""

import sys

if "/opt/trn_rl_repo" not in sys.path:
    sys.path.insert(0, "/opt/trn_rl_repo")

import numpy as np
import ml_dtypes

import concourse.bass as bass
import concourse.mybir as mybir
import concourse.tile as tile
from concourse.tile import ScopedClock
from concourse.bass_utils import run_bass_kernel_spmd

# ---------------- problem constants (hardcoded per contract) ----------------
N = 10000
E = 160000
MUL = 16
DIM_IN = 64
DIM_SH = 4
EMB = 18
HID = 256
WNUM = 1024

NCORES = 8
T = 512          # edges per tile
SUBT = 128       # edges per subtile (partition dim for scatter)
WIN = 256        # node window per group
GT = 4           # tiles per group
GROUP_E = GT * T

BF16 = ml_dtypes.bfloat16
F32 = np.float32

_BUILD_CACHE = {}
_last_exec_ns = None
_last_results = None


# --------------------------------------------------------------------------
# walrus's setupSyncWait in this container rejects instructions with more
# than one sem wait; Tile can emit several.  Two fixes: split the tail
# drain's waits across consecutive drains, and a post-pass that moves excess
# waits from any instruction onto same-engine NOPs placed right before it
# (identical semantics: waits AND together, the engine stalls either way).
# --------------------------------------------------------------------------
def _patched_drain_and_barrier(self, tick_clock, wait_clock):
    drain_inst = self.nc.sync.drain()
    wait_clock.add_sem_waits(
        drain_inst.ins, ScopedClock({None: tick_clock.global_clock})
    )
    si = drain_inst.ins.sync_info
    if si is not None and si.on_wait and len(si.on_wait) > 1:
        waits = list(si.on_wait)
        drain_inst.ins.sync_info = mybir.SyncInfo(
            on_wait=waits[:1], on_update=list(si.on_update or [])
        )
        for w in waits[1:]:
            extra = self.nc.sync.drain()
            extra.ins.sync_info = mybir.SyncInfo(on_wait=[w], on_update=[])
    self.nc.all_engine_barrier()
    assert self.sems is not None
    popped = self.nc._tile_sem_poison_stack.pop()
    assert popped is self._sem_poison
    self.nc.clear_and_free_semaphores(list(self.sems.allocated().values()))
    self.nc.all_engine_barrier()


if getattr(tile.TileContext, "_drain_patch", None) is not True:
    tile.TileContext._drain_and_barrier = _patched_drain_and_barrier
    tile.TileContext._drain_patch = True


def _split_excess_waits(nc: bass.Bass, maxw: int = 1) -> None:
    for f in nc.m.functions:
        for bb in f.blocks:
            rewritten = []
            changed = False
            for inst in bb.instructions:
                si = inst.sync_info
                if si is not None and si.on_wait and len(si.on_wait) > maxw:
                    waits = list(si.on_wait)
                    extra, keep = waits[:-maxw], waits[-maxw:]
                    for i, w in enumerate(extra):
                        nop = mybir.InstNoOp(
                            name=f"{inst.name}-ws{i}",
                            engine=inst.engine,
                            ins=[],
                            outs=[],
                        )
                        nop.sync_info = mybir.SyncInfo(on_wait=[w], on_update=[])
                        rewritten.append(nop)
                    inst.sync_info = mybir.SyncInfo(
                        on_wait=keep, on_update=list(si.on_update or [])
                    )
                    changed = True
                rewritten.append(inst)
            if changed:
                bb.instructions = rewritten


# ---------------------------- device program ------------------------------
def _build(ng: int, split_waits: bool = True, act: str = "silu") -> bass.Bass:
    """Build the per-core Bass program for ng groups (= ng*GT tiles)."""
    f32 = mybir.dt.float32
    f32r = mybir.dt.float32r
    bf16 = mybir.dt.bfloat16

    nt = ng * GT
    epc = nt * T
    nsub = epc // SUBT

    nc = bass.Bass(trn_type="TRN2")

    embT = nc.dram_tensor("embt", [EMB, epc], f32r, kind="ExternalInput")
    reps = nc.dram_tensor("reps", [128, 6, epc], bf16, kind="ExternalInput")
    featB = nc.dram_tensor("featb", [48, epc], bf16, kind="ExternalInput")
    oh = nc.dram_tensor("oh", [nsub, SUBT, WIN], bf16, kind="ExternalInput")
    w1 = nc.dram_tensor("w1", [EMB, HID], f32r, kind="ExternalInput")
    b1 = nc.dram_tensor("b1", [HID, 1], f32, kind="ExternalInput")
    w2p = nc.dram_tensor("w2p", [HID, WNUM], f32r, kind="ExternalInput")
    selab = nc.dram_tensor("selab", [128, 32], bf16, kind="ExternalInput")
    selv = nc.dram_tensor("selv", [128, 6, 48], bf16, kind="ExternalInput")
    s48 = nc.dram_tensor("s48", [16, 48], bf16, kind="ExternalInput")
    ident = nc.dram_tensor("ident", [64, 64], bf16, kind="ExternalInput")
    out = nc.dram_tensor("out", [ng, DIM_IN, WIN], f32, kind="ExternalOutput")

    with tile.TileContext(nc) as tc:
        with (
            tc.tile_pool(name="consts", bufs=1) as cpool,
            tc.tile_pool(name="io", bufs=3) as iop,
            tc.tile_pool(name="hbuf", bufs=2) as hpool,
            tc.tile_pool(name="wvbuf", bufs=2) as wvp,
            tc.tile_pool(name="prod", bufs=2) as prodp,
            tc.tile_pool(name="repsmall", bufs=3) as repsb,
            tc.tile_pool(name="efp", bufs=2) as efp,
            tc.tile_pool(name="winp", bufs=2) as winp,
            tc.tile_pool(name="psw", bufs=2, space="PSUM") as psw,
            tc.tile_pool(name="pssmall", bufs=1, space="PSUM") as pssmall,
            tc.tile_pool(name="psseg", bufs=1, space="PSUM") as psseg,
            tc.tile_pool(name="pstr", bufs=1, space="PSUM") as pstr,
        ):
            # ---- constants, loaded once ----
            w1_sb = cpool.tile([EMB, HID], f32r)
            nc.sync.dma_start(out=w1_sb[:], in_=w1[:])
            b1_sb = cpool.tile([128, 2], f32)
            nc.sync.dma_start(
                out=b1_sb[:], in_=b1.rearrange("(c p) o -> p (c o)", p=128)
            )
            w2p_sb = cpool.tile([128, 2, WNUM], f32r)
            nc.sync.dma_start(
                out=w2p_sb[:], in_=w2p.rearrange("(c p) m -> p c m", p=128)
            )
            selab_sb = cpool.tile([128, 32], bf16)
            nc.sync.dma_start(out=selab_sb[:], in_=selab[:])
            selv_sb = cpool.tile([128, 6, 48], bf16)
            nc.sync.dma_start(out=selv_sb[:], in_=selv[:])
            s48_sb = cpool.tile([16, 48], bf16)
            nc.sync.dma_start(out=s48_sb[:], in_=s48[:])
            ident_sb = cpool.tile([64, 64], bf16)
            nc.sync.dma_start(out=ident_sb[:], in_=ident[:])

            act_fn = (
                mybir.ActivationFunctionType.Silu
                if act == "silu"
                else mybir.ActivationFunctionType.Sigmoid
            )

            for g in range(ng):
                win_sb = winp.tile([64, WIN], f32, tag="win")
                for tt in range(GT):
                    t = g * GT + tt
                    tsl = slice(t * T, (t + 1) * T)
                    # ---------------- loads ----------------
                    emb_t = iop.tile([EMB, T], f32r, tag="embt")
                    nc.sync.dma_start(out=emb_t[:], in_=embT[:, tsl])
                    reps_t = iop.tile([128, 6, T], bf16, tag="reps")
                    nc.sync.dma_start(out=reps_t[:], in_=reps[:, :, tsl])
                    fb_t = iop.tile([48, T], bf16, tag="featb")
                    nc.sync.dma_start(out=fb_t[:], in_=featB[:, tsl])
                    oh_t = iop.tile([SUBT, 4, WIN], bf16, tag="oh")
                    nc.sync.dma_start(
                        out=oh_t[:],
                        in_=oh[4 * t : 4 * t + 4].rearrange("s p c -> p s c"),
                    )

                    # ---------------- MLP1: hT = silu(W1.T @ embT + b1) ----
                    hp_ps = psw.tile([128, 2, T], f32, tag="psw")
                    h_sb = []
                    for k in range(2):
                        nc.tensor.matmul(
                            hp_ps[:, k, :],
                            w1_sb[:, k * 128 : (k + 1) * 128],
                            emb_t[:],
                            start=True,
                            stop=True,
                        )
                        h_k = hpool.tile([128, T], f32r, tag=f"h{k}")
                        nc.scalar.activation(
                            h_k[:], hp_ps[:, k, :], act_fn,
                            bias=b1_sb[:, k : k + 1],
                        )
                        h_sb.append(h_k)

                    # ------- MLP2 (paired PSUM banks) + DVE products -------
                    # pair pm covers wT m-blocks {2pm, 2pm+1}; paths:
                    # pm0: ss (* sy0rep), pm1: vv (* crep), pm2: sv (* srep),
                    # pm3: vs (* vy0rep[a], a=0..2; via SBUF since reused 3x)
                    ppair = []
                    for pm in range(3):
                        wp_ps = psw.tile([128, 2, T], f32, tag="psw")
                        for half in range(2):
                            m = 2 * pm + half
                            for k in range(2):
                                nc.tensor.matmul(
                                    wp_ps[:, half, :],
                                    w2p_sb[:, k, m * 128 : (m + 1) * 128],
                                    h_sb[k][:],
                                    start=(k == 0),
                                    stop=(k == 1),
                                )
                        pp = prodp.tile([128, 2, T], bf16, tag=f"pp{pm}")
                        nc.vector.tensor_mul(
                            pp[:],
                            wp_ps[:],
                            reps_t[:, pm : pm + 1, :].to_broadcast([128, 2, T]),
                        )
                        ppair.append(pp)

                    wp3_ps = psw.tile([128, 2, T], f32, tag="psw")
                    for half in range(2):
                        m = 6 + half
                        for k in range(2):
                            nc.tensor.matmul(
                                wp3_ps[:, half, :],
                                w2p_sb[:, k, m * 128 : (m + 1) * 128],
                                h_sb[k][:],
                                start=(k == 0),
                                stop=(k == 1),
                            )
                    wv_sb = wvp.tile([128, 2, T], bf16, tag="wv")
                    nc.scalar.copy(wv_sb[:], wp3_ps[:])
                    p3 = []
                    for a in range(3):
                        pp = prodp.tile([128, 2, T], bf16, tag=f"p3{a}")
                        nc.vector.tensor_mul(
                            pp[:],
                            wv_sb[:],
                            reps_t[:, 3 + a : 4 + a, :].to_broadcast(
                                [128, 2, T]
                            ),
                        )
                        p3.append(pp)

                    # -------- PE column sums (contract over i) --------
                    small_ps = pssmall.tile([128, T], f32, tag="small")
                    small2_ps = pssmall.tile([64, T], f32, tag="small2")
                    os_ps = small_ps[0:16, :]
                    d_ps = small_ps[32:48, :]
                    drep_ps = small_ps[64:112, :]
                    ov_ps = small2_ps[0:48, :]

                    sel_a = selab_sb[:, 0:16]
                    sel_b = selab_sb[:, 16:32]
                    nc.tensor.matmul(os_ps, sel_a, ppair[0][:, 0, :], start=True, stop=False)
                    nc.tensor.matmul(os_ps, sel_a, ppair[1][:, 0, :], start=False, stop=False)
                    nc.tensor.matmul(os_ps, sel_b, ppair[0][:, 1, :], start=False, stop=False)
                    nc.tensor.matmul(os_ps, sel_b, ppair[1][:, 1, :], start=False, stop=True)

                    nc.tensor.matmul(d_ps, sel_a, ppair[2][:, 0, :], start=True, stop=False)
                    nc.tensor.matmul(d_ps, sel_b, ppair[2][:, 1, :], start=False, stop=True)
                    d_sb = repsb.tile([16, T], bf16, tag="dsb")
                    nc.scalar.copy(d_sb[:], d_ps)

                    nc.tensor.matmul(drep_ps, s48_sb[:], d_sb[:], start=True, stop=True)
                    drep_sb = repsb.tile([48, T], bf16, tag="drepsb")
                    nc.scalar.copy(drep_sb[:], drep_ps)

                    first = True
                    for b in range(2):
                        for a in range(3):
                            nc.tensor.matmul(
                                ov_ps,
                                selv_sb[:, b * 3 + a, :],
                                p3[a][:, b, :],
                                start=first,
                                stop=(b == 1 and a == 2),
                            )
                            first = False

                    # -------- edge features (16-row s part, 48-row v part) --
                    ef_s = efp.tile([16, T], bf16, tag="efs")
                    nc.scalar.copy(ef_s[:], os_ps)
                    ef_v = efp.tile([48, T], bf16, tag="efv")
                    tmpv = efp.tile([48, T], bf16, tag="tmpv")
                    nc.vector.tensor_mul(tmpv[:], drep_sb[:], fb_t[:])
                    nc.vector.tensor_add(ef_v[:], tmpv[:], ov_ps)

                    # -------- scatter: transpose + one-hot matmul --------
                    ef_rs = []
                    for sub in range(4):
                        tr_ps = pstr.tile([128, 64], bf16, tag="tr")
                        esl = slice(sub * 128, (sub + 1) * 128)
                        nc.tensor.transpose(
                            tr_ps[:, 0:16], ef_s[:, esl], ident_sb[0:16, 0:16]
                        )
                        nc.tensor.transpose(
                            tr_ps[:, 16:64], ef_v[:, esl], ident_sb[0:48, 0:48]
                        )
                        ef_r = efp.tile([128, 64], bf16, tag=f"efr{sub}")
                        nc.scalar.copy(ef_r[:], tr_ps[:])
                        ef_rs.append(ef_r)
                    # out[f, n] = sum_e ef_r[e, f] * oh[e, n]
                    seg_ps = psseg.tile([64, WIN], f32, tag="seg")
                    for sub in range(4):
                        nc.tensor.matmul(
                            seg_ps[:],
                            ef_rs[sub][:],
                            oh_t[:, sub, :],
                            start=(sub == 0),
                            stop=(sub == 3),
                        )

                    if tt == 0:
                        nc.scalar.copy(win_sb[:], seg_ps[:])
                    else:
                        nc.vector.tensor_add(win_sb[:], win_sb[:], seg_ps[:])

                nc.sync.dma_start(out=out[g], in_=win_sb[:])

    if split_waits:
        _split_excess_waits(nc)
    return nc


# ---------------------------- host-side prep ------------------------------
def _plan_groups(dst: np.ndarray):
    """Greedy cut into groups of <= GROUP_E edges spanning < WIN node ids."""
    groups = []
    i, n = 0, len(dst)
    while i < n:
        lo = int(dst[i])
        j = min(i + GROUP_E, int(np.searchsorted(dst, lo + WIN, side="left")))
        j = max(j, i + 1)
        groups.append((i, j, lo))
        i = j
    return groups


def _sel_constants():
    r = np.arange(128)
    selab = np.zeros((128, 32), BF16)
    selab[r, r // 16] = 1
    selab[r, 16 + r // 16 + 8] = 1
    selv = np.zeros((128, 6, 48), BF16)
    for b in range(2):
        for a in range(3):
            selv[r, b * 3 + a, (r // 16 + 8 * b) * 3 + a] = 1
    s48 = (np.arange(48)[None, :] // 3 == np.arange(16)[:, None]).astype(BF16)
    ident = np.eye(64, dtype=BF16)
    return selab, selv, s48, ident


def _permuted_w2(W2: np.ndarray) -> np.ndarray:
    inv3 = 1.0 / np.sqrt(3.0)
    c_s = np.sqrt(1.0 / (2 * MUL))
    c_v = np.sqrt(3.0 / (2 * MUL))
    z = 1.0 / np.sqrt(E / float(N))
    scale = np.array([c_s * z, c_s * inv3 * z, c_v * inv3 * z, c_v * inv3 * z])
    c = np.arange(WNUM)
    p, rem = c // 256, c % 256
    jj, ii = rem // 16, rem % 16
    src = p * 256 + ii * 16 + jj
    return (W2[:, src] * scale[p][None, :]).astype(F32)


def kernel(
    node_features,
    edge_attr,
    edge_embedding,
    W1,
    b1,
    W2,
    b2,
    edge_src,
    edge_dst,
):
    node_features = np.asarray(node_features, F32)
    edge_attr = np.asarray(edge_attr, F32)
    edge_embedding = np.asarray(edge_embedding, F32)
    W1 = np.asarray(W1, F32)
    b1 = np.asarray(b1, F32)
    W2 = np.asarray(W2, F32)
    b2 = np.asarray(b2, F32)
    edge_src = np.asarray(edge_src)
    edge_dst = np.asarray(edge_dst)

    order = np.argsort(edge_dst, kind="stable")
    per_core = E // NCORES

    core_groups = []
    for c in range(NCORES):
        o = order[c * per_core : (c + 1) * per_core]
        core_groups.append((o, _plan_groups(edge_dst[o])))
    ng = max(len(g) for _, g in core_groups)

    nc = _BUILD_CACHE.get(ng)
    if nc is None:
        nc = _build(ng)
        _BUILD_CACHE[ng] = nc

    nt = ng * GT
    epc = nt * T
    nsub = epc // SUBT
    w2p = _permuted_w2(W2)
    selab, selv, s48, ident = _sel_constants()
    ridx = np.arange(128) % 16  # partition r holds feature row r%16

    in_maps = []
    los_all = []
    for c in range(NCORES):
        o, groups = core_groups[c]
        perm = np.full(epc, -1, np.int64)
        los = np.zeros(ng, np.int64)
        col = np.zeros(epc, np.int64)
        for g, (i, j, lo) in enumerate(groups):
            k = j - i
            perm[g * GROUP_E : g * GROUP_E + k] = o[i:j]
            los[g] = lo
            col[g * GROUP_E : g * GROUP_E + k] = edge_dst[o[i:j]] - lo
        valid = perm >= 0
        pidx = np.where(valid, perm, 0)

        emb_p = np.where(valid[:, None], edge_embedding[pidx], 0.0).astype(F32)
        x = node_features[edge_src[pidx]]
        y = edge_attr[pidx]
        s = x[:, :MUL]
        v3 = x[:, MUL:].reshape(-1, MUL, 3)
        y0 = y[:, 0:1]
        y1 = y[:, 1:4]
        sy0 = s * y0
        cfeat = (v3 * y1[:, None, :]).sum(-1)
        vy0a = (v3 * y0[:, :, None]).transpose(0, 2, 1).reshape(-1, 48)
        # modulators, already replicated to the 128-partition layout
        # (partition r = jj*16 + i holds feature column i = r%16)
        mods = np.empty((epc, 6, 16), F32)
        mods[:, 0] = sy0
        mods[:, 1] = cfeat
        mods[:, 2] = s
        mods[:, 3] = vy0a[:, 0:16]
        mods[:, 4] = vy0a[:, 16:32]
        mods[:, 5] = vy0a[:, 32:48]
        mods[~valid] = 0.0
        reps_arr = np.ascontiguousarray(
            mods[:, :, ridx].transpose(2, 1, 0).astype(BF16)
        )  # [128, 6, epc]

        feat_b = np.tile(y1, (1, MUL))
        feat_b[~valid] = 0.0

        ohm = np.zeros((epc, WIN), BF16)
        ohm[np.where(valid)[0], col[valid]] = 1

        in_maps.append(
            dict(
                embt=np.ascontiguousarray(emb_p.T),
                reps=reps_arr,
                featb=np.ascontiguousarray(feat_b.T.astype(BF16)),
                oh=ohm.reshape(nsub, SUBT, WIN),
                w1=W1,
                b1=b1.reshape(HID, 1),
                w2p=w2p,
                selab=selab,
                selv=selv,
                s48=s48,
                ident=ident,
            )
        )
        los_all.append(los)

    res = run_bass_kernel_spmd(nc, in_maps, core_ids=list(range(NCORES)))
    global _last_exec_ns, _last_results
    _last_exec_ns = res.exec_time_ns
    _last_results = res

    out = np.zeros((N, DIM_IN), F32)
    for c in range(NCORES):
        blocks = res.results[c]["out"]  # [ng, 64, WIN]
        los = los_all[c]
        ngc = len(core_groups[c][1])
        for g in range(ngc):
            lo = int(los[g])
            hi = min(lo + WIN, N)
            out[lo:hi] += blocks[g].T[: hi - lo]

    if np.any(b2):
        out += _b2_contribution(node_features, edge_attr, b2, edge_src, edge_dst)
    return out


def _b2_contribution(node_features, edge_attr, b2, edge_src, edge_dst):
    inv3 = 1.0 / np.sqrt(3.0)
    c_s = np.sqrt(1.0 / (2 * MUL))
    c_v = np.sqrt(3.0 / (2 * MUL))
    z = 1.0 / np.sqrt(E / float(N))
    w = b2.reshape(4, MUL, MUL)
    x = node_features[edge_src]
    s = x[:, :MUL]
    v = x[:, MUL:].reshape(-1, MUL, 3)
    y0 = edge_attr[:, 0]
    y1 = edge_attr[:, 1:4]
    out_s = c_s * (
        np.einsum("ij,e,ei->ej", w[0], y0, s)
        + inv3 * np.einsum("ij,ea,eia->ej", w[1], y1, v)
    )
    out_v = (c_v * inv3) * (
        np.einsum("ij,ei,ea->eja", w[2], s, y1)
        + np.einsum("ij,eia,e->eja", w[3], v, y0)
    )
    ef = np.concatenate([out_s, out_v.reshape(-1, 3 * MUL)], axis=1)
    out = np.zeros((N, DIM_IN), F32)
    np.add.at(out, edge_dst, ef)
    return (out * z).astype(F32)


# revision 14
# speedup vs baseline: 1.6275x; 1.1378x over previous
"""NequIP-style GNN message passing layer on 8 Trainium2 NeuronCores.

Strategy (edges partitioned across cores per the sharding hint):
  * Host: sort edges by destination, split into 8 contiguous shards, gather
    source-node features per edge, precompute the per-edge tensor-product
    modulators already replicated across the 128-partition layout the device
    needs (sy0, C, s, v*y0 per spherical component), pad edges into
    fixed-size groups whose destinations span < 256 node ids, and build
    per-group one-hot scatter matrices.  All TP path constants and the
    1/sqrt(deg) normalization are folded into a column-permuted copy of W2
    (j-major inside each 16x16 block so the i-contraction is a contiguous
    16-partition column sum).
  * Device (per 512-edge tile; features on partitions, edges on free dim):
      - MLP1/MLP2 as fp32r matmuls -> per-edge TP weights wT [1024, T] in
        paired PSUM banks [128, 2, T]
      - DVE elementwise products w * modulator, mostly straight out of PSUM
      - PE column-sum matmuls with constant 0/1 selection matrices contract
        over the multiplicity index i
      - edge features are PE-transposed to edges-on-partitions and
        scatter-summed via one-hot bf16 matmuls into a [64, 256] window
        accumulated in SBUF, DMA'd out per group (host transposes windows)
  * Host: add the per-window partial sums from all cores into [10000, 64].
"""

import sys

if "/opt/trn_rl_repo" not in sys.path:
    sys.path.insert(0, "/opt/trn_rl_repo")

import numpy as np
import ml_dtypes

import concourse.bass as bass
import concourse.mybir as mybir
import concourse.tile as tile
from concourse.tile import ScopedClock
from concourse.bass_utils import run_bass_kernel_spmd

# ---------------- problem constants (hardcoded per contract) ----------------
N = 10000
E = 160000
MUL = 16
DIM_IN = 64
DIM_SH = 4
EMB = 18
HID = 256
WNUM = 1024

NCORES = 8
T = 512          # edges per tile
SUBT = 128       # edges per subtile (partition dim for scatter)
WIN = 256        # node window per group
GT = 4           # tiles per group
GROUP_E = GT * T

BF16 = ml_dtypes.bfloat16
F32 = np.float32

_BUILD_CACHE = {}
_last_exec_ns = None
_last_results = None


# --------------------------------------------------------------------------
# walrus's setupSyncWait in this container rejects instructions with more
# than one sem wait; Tile can emit several.  Two fixes: split the tail
# drain's waits across consecutive drains, and a post-pass that moves excess
# waits from any instruction onto same-engine NOPs placed right before it
# (identical semantics: waits AND together, the engine stalls either way).
# --------------------------------------------------------------------------
def _patched_drain_and_barrier(self, tick_clock, wait_clock):
    drain_inst = self.nc.sync.drain()
    wait_clock.add_sem_waits(
        drain_inst.ins, ScopedClock({None: tick_clock.global_clock})
    )
    si = drain_inst.ins.sync_info
    if si is not None and si.on_wait and len(si.on_wait) > 1:
        waits = list(si.on_wait)
        drain_inst.ins.sync_info = mybir.SyncInfo(
            on_wait=waits[:1], on_update=list(si.on_update or [])
        )
        for w in waits[1:]:
            extra = self.nc.sync.drain()
            extra.ins.sync_info = mybir.SyncInfo(on_wait=[w], on_update=[])
    self.nc.all_engine_barrier()
    assert self.sems is not None
    popped = self.nc._tile_sem_poison_stack.pop()
    assert popped is self._sem_poison
    self.nc.clear_and_free_semaphores(list(self.sems.allocated().values()))
    self.nc.all_engine_barrier()


if getattr(tile.TileContext, "_drain_patch", None) is not True:
    tile.TileContext._drain_and_barrier = _patched_drain_and_barrier
    tile.TileContext._drain_patch = True


def _split_excess_waits(nc: bass.Bass, maxw: int = 1) -> None:
    for f in nc.m.functions:
        for bb in f.blocks:
            rewritten = []
            changed = False
            for inst in bb.instructions:
                si = inst.sync_info
                if si is not None and si.on_wait and len(si.on_wait) > maxw:
                    waits = list(si.on_wait)
                    extra, keep = waits[:-maxw], waits[-maxw:]
                    for i, w in enumerate(extra):
                        nop = mybir.InstNoOp(
                            name=f"{inst.name}-ws{i}",
                            engine=inst.engine,
                            ins=[],
                            outs=[],
                        )
                        nop.sync_info = mybir.SyncInfo(on_wait=[w], on_update=[])
                        rewritten.append(nop)
                    inst.sync_info = mybir.SyncInfo(
                        on_wait=keep, on_update=list(si.on_update or [])
                    )
                    changed = True
                rewritten.append(inst)
            if changed:
                bb.instructions = rewritten


# ---------------------------- device program ------------------------------
def _build(ng: int, split_waits: bool = True, act: str = "silu") -> bass.Bass:
    """Build the per-core Bass program for ng groups (= ng*GT tiles)."""
    f32 = mybir.dt.float32
    f32r = mybir.dt.float32r
    bf16 = mybir.dt.bfloat16

    nt = ng * GT
    epc = nt * T
    nsub = epc // SUBT

    nc = bass.Bass(trn_type="TRN2")

    embT = nc.dram_tensor("embt", [EMB, epc], bf16, kind="ExternalInput")
    reps = nc.dram_tensor("reps", [128, 6, epc], bf16, kind="ExternalInput")
    featB = nc.dram_tensor("featb", [48, epc], bf16, kind="ExternalInput")
    oh = nc.dram_tensor("oh", [nsub, SUBT, WIN], bf16, kind="ExternalInput")
    w1 = nc.dram_tensor("w1", [EMB, HID], bf16, kind="ExternalInput")
    b1 = nc.dram_tensor("b1", [HID, 1], f32, kind="ExternalInput")
    w2p = nc.dram_tensor("w2p", [HID, WNUM], bf16, kind="ExternalInput")
    selab = nc.dram_tensor("selab", [128, 32], bf16, kind="ExternalInput")
    selv = nc.dram_tensor("selv", [128, 6, 48], bf16, kind="ExternalInput")
    s48 = nc.dram_tensor("s48", [16, 48], bf16, kind="ExternalInput")
    ident = nc.dram_tensor("ident", [64, 64], bf16, kind="ExternalInput")
    out = nc.dram_tensor("out", [ng, DIM_IN, WIN], f32, kind="ExternalOutput")

    with tile.TileContext(nc) as tc:
        with (
            tc.tile_pool(name="consts", bufs=1) as cpool,
            tc.tile_pool(name="io", bufs=3) as iop,
            tc.tile_pool(name="hbuf", bufs=2) as hpool,
            tc.tile_pool(name="wvbuf", bufs=2) as wvp,
            tc.tile_pool(name="prod", bufs=2) as prodp,
            tc.tile_pool(name="repsmall", bufs=3) as repsb,
            tc.tile_pool(name="efp", bufs=2) as efp,
            tc.tile_pool(name="winp", bufs=2) as winp,
            tc.tile_pool(name="psw", bufs=2, space="PSUM") as psw,
            tc.tile_pool(name="pssmall", bufs=1, space="PSUM") as pssmall,
            tc.tile_pool(name="psseg", bufs=1, space="PSUM") as psseg,
            tc.tile_pool(name="pstr", bufs=1, space="PSUM") as pstr,
        ):
            # ---- constants, loaded once ----
            w1_sb = cpool.tile([EMB, HID], bf16)
            nc.sync.dma_start(out=w1_sb[:], in_=w1[:])
            b1_sb = cpool.tile([128, 2], f32)
            nc.sync.dma_start(
                out=b1_sb[:], in_=b1.rearrange("(c p) o -> p (c o)", p=128)
            )
            w2p_sb = cpool.tile([128, 2, WNUM], bf16)
            nc.sync.dma_start(
                out=w2p_sb[:], in_=w2p.rearrange("(c p) m -> p c m", p=128)
            )
            selab_sb = cpool.tile([128, 32], bf16)
            nc.sync.dma_start(out=selab_sb[:], in_=selab[:])
            selv_sb = cpool.tile([128, 6, 48], bf16)
            nc.sync.dma_start(out=selv_sb[:], in_=selv[:])
            s48_sb = cpool.tile([16, 48], bf16)
            nc.sync.dma_start(out=s48_sb[:], in_=s48[:])
            ident_sb = cpool.tile([64, 64], bf16)
            nc.sync.dma_start(out=ident_sb[:], in_=ident[:])

            act_fn = (
                mybir.ActivationFunctionType.Silu
                if act == "silu"
                else mybir.ActivationFunctionType.Sigmoid
            )

            for g in range(ng):
                win_sb = winp.tile([64, WIN], f32, tag="win")
                for tt in range(GT):
                    t = g * GT + tt
                    tsl = slice(t * T, (t + 1) * T)
                    # ---------------- loads ----------------
                    emb_t = iop.tile([EMB, T], bf16, tag="embt")
                    nc.sync.dma_start(out=emb_t[:], in_=embT[:, tsl])
                    reps_t = iop.tile([128, 6, T], bf16, tag="reps")
                    nc.sync.dma_start(out=reps_t[:], in_=reps[:, :, tsl])
                    fb_t = iop.tile([48, T], bf16, tag="featb")
                    nc.sync.dma_start(out=fb_t[:], in_=featB[:, tsl])
                    oh_t = iop.tile([SUBT, 4, WIN], bf16, tag="oh")
                    nc.sync.dma_start(
                        out=oh_t[:],
                        in_=oh[4 * t : 4 * t + 4].rearrange("s p c -> p s c"),
                    )

                    # ---------------- MLP1: hT = silu(W1.T @ embT + b1) ----
                    hp_ps = psw.tile([128, 2, T], f32, tag="psw")
                    h_sb = []
                    for k in range(2):
                        nc.tensor.matmul(
                            hp_ps[:, k, :],
                            w1_sb[:, k * 128 : (k + 1) * 128],
                            emb_t[:],
                            start=True,
                            stop=True,
                        )
                        h_k = hpool.tile([128, T], bf16, tag=f"h{k}")
                        nc.scalar.activation(
                            h_k[:], hp_ps[:, k, :], act_fn,
                            bias=b1_sb[:, k : k + 1],
                        )
                        h_sb.append(h_k)

                    # ------- MLP2 (paired PSUM banks) + DVE products -------
                    # pair pm covers wT m-blocks {2pm, 2pm+1}; paths:
                    # pm0: ss (* sy0rep), pm1: vv (* crep), pm2: sv (* srep),
                    # pm3: vs (* vy0rep[a], a=0..2; via SBUF since reused 3x)
                    ppair = []
                    for pm in range(3):
                        wp_ps = psw.tile([128, 2, T], f32, tag="psw")
                        for half in range(2):
                            m = 2 * pm + half
                            for k in range(2):
                                nc.tensor.matmul(
                                    wp_ps[:, half, :],
                                    w2p_sb[:, k, m * 128 : (m + 1) * 128],
                                    h_sb[k][:],
                                    start=(k == 0),
                                    stop=(k == 1),
                                )
                        pp = prodp.tile([128, 2, T], bf16, tag=f"pp{pm}")
                        nc.vector.tensor_mul(
                            pp[:],
                            wp_ps[:],
                            reps_t[:, pm : pm + 1, :].to_broadcast([128, 2, T]),
                        )
                        ppair.append(pp)

                    wp3_ps = psw.tile([128, 2, T], f32, tag="psw")
                    for half in range(2):
                        m = 6 + half
                        for k in range(2):
                            nc.tensor.matmul(
                                wp3_ps[:, half, :],
                                w2p_sb[:, k, m * 128 : (m + 1) * 128],
                                h_sb[k][:],
                                start=(k == 0),
                                stop=(k == 1),
                            )
                    wv_sb = wvp.tile([128, 2, T], bf16, tag="wv")
                    nc.scalar.copy(wv_sb[:], wp3_ps[:])
                    p3 = []
                    for a in range(3):
                        pp = prodp.tile([128, 2, T], bf16, tag=f"p3{a}")
                        nc.vector.tensor_mul(
                            pp[:],
                            wv_sb[:],
                            reps_t[:, 3 + a : 4 + a, :].to_broadcast(
                                [128, 2, T]
                            ),
                        )
                        p3.append(pp)

                    # -------- PE column sums (contract over i) --------
                    small_ps = pssmall.tile([128, T], f32, tag="small")
                    small2_ps = pssmall.tile([64, T], f32, tag="small2")
                    os_ps = small_ps[0:16, :]
                    d_ps = small_ps[32:48, :]
                    drep_ps = small_ps[64:112, :]
                    ov_ps = small2_ps[0:48, :]

                    sel_a = selab_sb[:, 0:16]
                    sel_b = selab_sb[:, 16:32]
                    nc.tensor.matmul(os_ps, sel_a, ppair[0][:, 0, :], start=True, stop=False)
                    nc.tensor.matmul(os_ps, sel_a, ppair[1][:, 0, :], start=False, stop=False)
                    nc.tensor.matmul(os_ps, sel_b, ppair[0][:, 1, :], start=False, stop=False)
                    nc.tensor.matmul(os_ps, sel_b, ppair[1][:, 1, :], start=False, stop=True)

                    nc.tensor.matmul(d_ps, sel_a, ppair[2][:, 0, :], start=True, stop=False)
                    nc.tensor.matmul(d_ps, sel_b, ppair[2][:, 1, :], start=False, stop=True)
                    d_sb = repsb.tile([16, T], bf16, tag="dsb")
                    nc.scalar.copy(d_sb[:], d_ps)

                    nc.tensor.matmul(drep_ps, s48_sb[:], d_sb[:], start=True, stop=True)
                    drep_sb = repsb.tile([48, T], bf16, tag="drepsb")
                    nc.scalar.copy(drep_sb[:], drep_ps)

                    first = True
                    for b in range(2):
                        for a in range(3):
                            nc.tensor.matmul(
                                ov_ps,
                                selv_sb[:, b * 3 + a, :],
                                p3[a][:, b, :],
                                start=first,
                                stop=(b == 1 and a == 2),
                            )
                            first = False

                    # -------- edge features (16-row s part, 48-row v part) --
                    ef_s = efp.tile([16, T], bf16, tag="efs")
                    nc.scalar.copy(ef_s[:], os_ps)
                    ef_v = efp.tile([48, T], bf16, tag="efv")
                    tmpv = efp.tile([48, T], bf16, tag="tmpv")
                    nc.vector.tensor_mul(tmpv[:], drep_sb[:], fb_t[:])
                    nc.vector.tensor_add(ef_v[:], tmpv[:], ov_ps)

                    # -------- scatter: transpose + one-hot matmul --------
                    ef_rs = []
                    for sub in range(4):
                        tr_ps = pstr.tile([128, 64], bf16, tag="tr")
                        esl = slice(sub * 128, (sub + 1) * 128)
                        nc.tensor.transpose(
                            tr_ps[:, 0:16], ef_s[:, esl], ident_sb[0:16, 0:16]
                        )
                        nc.tensor.transpose(
                            tr_ps[:, 16:64], ef_v[:, esl], ident_sb[0:48, 0:48]
                        )
                        ef_r = efp.tile([128, 64], bf16, tag=f"efr{sub}")
                        nc.scalar.copy(ef_r[:], tr_ps[:])
                        ef_rs.append(ef_r)
                    # out[f, n] = sum_e ef_r[e, f] * oh[e, n]
                    seg_ps = psseg.tile([64, WIN], f32, tag="seg")
                    for sub in range(4):
                        nc.tensor.matmul(
                            seg_ps[:],
                            ef_rs[sub][:],
                            oh_t[:, sub, :],
                            start=(sub == 0),
                            stop=(sub == 3),
                        )

                    if tt == 0:
                        nc.scalar.copy(win_sb[:], seg_ps[:])
                    else:
                        nc.vector.tensor_add(win_sb[:], win_sb[:], seg_ps[:])

                nc.sync.dma_start(out=out[g], in_=win_sb[:])

    if split_waits:
        _split_excess_waits(nc)
    return nc


# ---------------------------- host-side prep ------------------------------
def _plan_groups(dst: np.ndarray):
    """Greedy cut into groups of <= GROUP_E edges spanning < WIN node ids."""
    groups = []
    i, n = 0, len(dst)
    while i < n:
        lo = int(dst[i])
        j = min(i + GROUP_E, int(np.searchsorted(dst, lo + WIN, side="left")))
        j = max(j, i + 1)
        groups.append((i, j, lo))
        i = j
    return groups


def _sel_constants():
    r = np.arange(128)
    selab = np.zeros((128, 32), BF16)
    selab[r, r // 16] = 1
    selab[r, 16 + r // 16 + 8] = 1
    selv = np.zeros((128, 6, 48), BF16)
    for b in range(2):
        for a in range(3):
            selv[r, b * 3 + a, (r // 16 + 8 * b) * 3 + a] = 1
    s48 = (np.arange(48)[None, :] // 3 == np.arange(16)[:, None]).astype(BF16)
    ident = np.eye(64, dtype=BF16)
    return selab, selv, s48, ident


def _permuted_w2(W2: np.ndarray) -> np.ndarray:
    inv3 = 1.0 / np.sqrt(3.0)
    c_s = np.sqrt(1.0 / (2 * MUL))
    c_v = np.sqrt(3.0 / (2 * MUL))
    z = 1.0 / np.sqrt(E / float(N))
    scale = np.array([c_s * z, c_s * inv3 * z, c_v * inv3 * z, c_v * inv3 * z])
    c = np.arange(WNUM)
    p, rem = c // 256, c % 256
    jj, ii = rem // 16, rem % 16
    src = p * 256 + ii * 16 + jj
    return (W2[:, src] * scale[p][None, :]).astype(F32)


def kernel(
    node_features,
    edge_attr,
    edge_embedding,
    W1,
    b1,
    W2,
    b2,
    edge_src,
    edge_dst,
):
    node_features = np.asarray(node_features, F32)
    edge_attr = np.asarray(edge_attr, F32)
    edge_embedding = np.asarray(edge_embedding, F32)
    W1 = np.asarray(W1, F32)
    b1 = np.asarray(b1, F32)
    W2 = np.asarray(W2, F32)
    b2 = np.asarray(b2, F32)
    edge_src = np.asarray(edge_src)
    edge_dst = np.asarray(edge_dst)

    order = np.argsort(edge_dst, kind="stable")
    per_core = E // NCORES

    core_groups = []
    for c in range(NCORES):
        o = order[c * per_core : (c + 1) * per_core]
        core_groups.append((o, _plan_groups(edge_dst[o])))
    ng = max(len(g) for _, g in core_groups)

    nc = _BUILD_CACHE.get(ng)
    if nc is None:
        nc = _build(ng)
        _BUILD_CACHE[ng] = nc

    nt = ng * GT
    epc = nt * T
    nsub = epc // SUBT
    w2p = _permuted_w2(W2)
    selab, selv, s48, ident = _sel_constants()
    ridx = np.arange(128) % 16  # partition r holds feature row r%16

    in_maps = []
    los_all = []
    for c in range(NCORES):
        o, groups = core_groups[c]
        perm = np.full(epc, -1, np.int64)
        los = np.zeros(ng, np.int64)
        col = np.zeros(epc, np.int64)
        for g, (i, j, lo) in enumerate(groups):
            k = j - i
            perm[g * GROUP_E : g * GROUP_E + k] = o[i:j]
            los[g] = lo
            col[g * GROUP_E : g * GROUP_E + k] = edge_dst[o[i:j]] - lo
        valid = perm >= 0
        pidx = np.where(valid, perm, 0)

        emb_p = np.where(valid[:, None], edge_embedding[pidx], 0.0).astype(F32)
        x = node_features[edge_src[pidx]]
        y = edge_attr[pidx]
        s = x[:, :MUL]
        v3 = x[:, MUL:].reshape(-1, MUL, 3)
        y0 = y[:, 0:1]
        y1 = y[:, 1:4]
        sy0 = s * y0
        cfeat = (v3 * y1[:, None, :]).sum(-1)
        vy0a = (v3 * y0[:, :, None]).transpose(0, 2, 1).reshape(-1, 48)
        # modulators, already replicated to the 128-partition layout
        # (partition r = jj*16 + i holds feature column i = r%16)
        mods = np.empty((epc, 6, 16), F32)
        mods[:, 0] = sy0
        mods[:, 1] = cfeat
        mods[:, 2] = s
        mods[:, 3] = vy0a[:, 0:16]
        mods[:, 4] = vy0a[:, 16:32]
        mods[:, 5] = vy0a[:, 32:48]
        mods[~valid] = 0.0
        reps_arr = np.ascontiguousarray(
            mods[:, :, ridx].transpose(2, 1, 0).astype(BF16)
        )  # [128, 6, epc]

        feat_b = np.tile(y1, (1, MUL))
        feat_b[~valid] = 0.0

        ohm = np.zeros((epc, WIN), BF16)
        ohm[np.where(valid)[0], col[valid]] = 1

        in_maps.append(
            dict(
                embt=np.ascontiguousarray(emb_p.T.astype(BF16)),
                reps=reps_arr,
                featb=np.ascontiguousarray(feat_b.T.astype(BF16)),
                oh=ohm.reshape(nsub, SUBT, WIN),
                w1=W1.astype(BF16),
                b1=b1.reshape(HID, 1),
                w2p=w2p.astype(BF16),
                selab=selab,
                selv=selv,
                s48=s48,
                ident=ident,
            )
        )
        los_all.append(los)

    res = run_bass_kernel_spmd(nc, in_maps, core_ids=list(range(NCORES)))
    global _last_exec_ns, _last_results
    _last_exec_ns = res.exec_time_ns
    _last_results = res

    out = np.zeros((N, DIM_IN), F32)
    for c in range(NCORES):
        blocks = res.results[c]["out"]  # [ng, 64, WIN]
        los = los_all[c]
        ngc = len(core_groups[c][1])
        for g in range(ngc):
            lo = int(los[g])
            hi = min(lo + WIN, N)
            out[lo:hi] += blocks[g].T[: hi - lo]

    if np.any(b2):
        out += _b2_contribution(node_features, edge_attr, b2, edge_src, edge_dst)
    return out


def _b2_contribution(node_features, edge_attr, b2, edge_src, edge_dst):
    inv3 = 1.0 / np.sqrt(3.0)
    c_s = np.sqrt(1.0 / (2 * MUL))
    c_v = np.sqrt(3.0 / (2 * MUL))
    z = 1.0 / np.sqrt(E / float(N))
    w = b2.reshape(4, MUL, MUL)
    x = node_features[edge_src]
    s = x[:, :MUL]
    v = x[:, MUL:].reshape(-1, MUL, 3)
    y0 = edge_attr[:, 0]
    y1 = edge_attr[:, 1:4]
    out_s = c_s * (
        np.einsum("ij,e,ei->ej", w[0], y0, s)
        + inv3 * np.einsum("ij,ea,eia->ej", w[1], y1, v)
    )
    out_v = (c_v * inv3) * (
        np.einsum("ij,ei,ea->eja", w[2], s, y1)
        + np.einsum("ij,eia,e->eja", w[3], v, y0)
    )
    ef = np.concatenate([out_s, out_v.reshape(-1, 3 * MUL)], axis=1)
    out = np.zeros((N, DIM_IN), F32)
    np.add.at(out, edge_dst, ef)
    return (out * z).astype(F32)


# revision 16
# speedup vs baseline: 1.6905x; 1.0387x over previous
"""NequIP-style GNN message passing layer on 8 Trainium2 NeuronCores.

Strategy (edges partitioned across cores per the sharding hint):
  * Host: sort edges by destination, split into 8 contiguous shards, gather
    source-node features per edge, precompute the per-edge tensor-product
    modulators already replicated across the 128-partition layout the device
    needs (sy0, C, s, v*y0 per spherical component), pad edges into
    fixed-size groups whose destinations span < 256 node ids, and build
    per-group one-hot scatter matrices.  All TP path constants and the
    1/sqrt(deg) normalization are folded into a column-permuted copy of W2
    (j-major inside each 16x16 block so the i-contraction is a contiguous
    16-partition column sum).
  * Device (per 512-edge tile; features on partitions, edges on free dim):
      - MLP1/MLP2 as fp32r matmuls -> per-edge TP weights wT [1024, T] in
        paired PSUM banks [128, 2, T]
      - DVE elementwise products w * modulator, mostly straight out of PSUM
      - PE column-sum matmuls with constant 0/1 selection matrices contract
        over the multiplicity index i
      - edge features are PE-transposed to edges-on-partitions and
        scatter-summed via one-hot bf16 matmuls into a [64, 256] window
        accumulated in SBUF, DMA'd out per group (host transposes windows)
  * Host: add the per-window partial sums from all cores into [10000, 64].
"""

import sys

if "/opt/trn_rl_repo" not in sys.path:
    sys.path.insert(0, "/opt/trn_rl_repo")

import numpy as np
import ml_dtypes

import concourse.bass as bass
import concourse.mybir as mybir
import concourse.tile as tile
from concourse.tile import ScopedClock
from concourse.bass_utils import run_bass_kernel_spmd
from concourse import bass_utils as _bass_utils

if getattr(_bass_utils, "_ldw_patch", None) is not True:
    import os as _os

    if _os.environ.get("KERNEL_LDW_OPT") == "1":
        _orig_run_command = _bass_utils.run_command

        def _run_command_ldw(argv, **kw):
            argv = [
                "--enable-ldw-opt=true" if a == "--enable-ldw-opt=false" else a
                for a in argv
            ]
            return _orig_run_command(argv, **kw)

        _bass_utils.run_command = _run_command_ldw
    _bass_utils._ldw_patch = True

# ---------------- problem constants (hardcoded per contract) ----------------
N = 10000
E = 160000
MUL = 16
DIM_IN = 64
DIM_SH = 4
EMB = 18
HID = 256
WNUM = 1024

NCORES = 8
T = 512          # edges per tile
SUBT = 128       # edges per subtile (partition dim for scatter)
WIN = 256        # node window per group
GT = 4           # tiles per group
GROUP_E = GT * T

BF16 = ml_dtypes.bfloat16
F32 = np.float32

_BUILD_CACHE = {}
_last_exec_ns = None
_last_results = None


# --------------------------------------------------------------------------
# walrus's setupSyncWait in this container rejects instructions with more
# than one sem wait; Tile can emit several.  Two fixes: split the tail
# drain's waits across consecutive drains, and a post-pass that moves excess
# waits from any instruction onto same-engine NOPs placed right before it
# (identical semantics: waits AND together, the engine stalls either way).
# --------------------------------------------------------------------------
def _patched_drain_and_barrier(self, tick_clock, wait_clock):
    drain_inst = self.nc.sync.drain()
    wait_clock.add_sem_waits(
        drain_inst.ins, ScopedClock({None: tick_clock.global_clock})
    )
    si = drain_inst.ins.sync_info
    if si is not None and si.on_wait and len(si.on_wait) > 1:
        waits = list(si.on_wait)
        drain_inst.ins.sync_info = mybir.SyncInfo(
            on_wait=waits[:1], on_update=list(si.on_update or [])
        )
        for w in waits[1:]:
            extra = self.nc.sync.drain()
            extra.ins.sync_info = mybir.SyncInfo(on_wait=[w], on_update=[])
    self.nc.all_engine_barrier()
    assert self.sems is not None
    popped = self.nc._tile_sem_poison_stack.pop()
    assert popped is self._sem_poison
    self.nc.clear_and_free_semaphores(list(self.sems.allocated().values()))
    self.nc.all_engine_barrier()


if getattr(tile.TileContext, "_drain_patch", None) is not True:
    tile.TileContext._drain_and_barrier = _patched_drain_and_barrier
    tile.TileContext._drain_patch = True


def _split_excess_waits(nc: bass.Bass, maxw: int = 1) -> None:
    for f in nc.m.functions:
        for bb in f.blocks:
            rewritten = []
            changed = False
            for inst in bb.instructions:
                si = inst.sync_info
                if si is not None and si.on_wait and len(si.on_wait) > maxw:
                    waits = list(si.on_wait)
                    extra, keep = waits[:-maxw], waits[-maxw:]
                    for i, w in enumerate(extra):
                        nop = mybir.InstNoOp(
                            name=f"{inst.name}-ws{i}",
                            engine=inst.engine,
                            ins=[],
                            outs=[],
                        )
                        nop.sync_info = mybir.SyncInfo(on_wait=[w], on_update=[])
                        rewritten.append(nop)
                    inst.sync_info = mybir.SyncInfo(
                        on_wait=keep, on_update=list(si.on_update or [])
                    )
                    changed = True
                rewritten.append(inst)
            if changed:
                bb.instructions = rewritten


# ---------------------------- device program ------------------------------
def _build(ng: int, split_waits: bool = True, act: str = "silu") -> bass.Bass:
    """Build the per-core Bass program for ng groups (= ng*GT tiles)."""
    f32 = mybir.dt.float32
    f32r = mybir.dt.float32r
    bf16 = mybir.dt.bfloat16

    nt = ng * GT
    epc = nt * T
    nsub = epc // SUBT

    nc = bass.Bass(trn_type="TRN2")

    embT = nc.dram_tensor("embt", [EMB, epc], bf16, kind="ExternalInput")
    reps = nc.dram_tensor("reps", [128, 6, epc], bf16, kind="ExternalInput")
    featB = nc.dram_tensor("featb", [48, epc], bf16, kind="ExternalInput")
    oh = nc.dram_tensor("oh", [nsub, SUBT, WIN], bf16, kind="ExternalInput")
    w1 = nc.dram_tensor("w1", [EMB, HID], bf16, kind="ExternalInput")
    b1 = nc.dram_tensor("b1", [HID, 1], f32, kind="ExternalInput")
    w2p = nc.dram_tensor("w2p", [HID, WNUM], bf16, kind="ExternalInput")
    selab = nc.dram_tensor("selab", [128, 32], bf16, kind="ExternalInput")
    selv = nc.dram_tensor("selv", [128, 6, 48], bf16, kind="ExternalInput")
    sel48 = nc.dram_tensor("sel48", [128, 2, 48], bf16, kind="ExternalInput")
    ident = nc.dram_tensor("ident", [64, 64], bf16, kind="ExternalInput")
    out = nc.dram_tensor("out", [ng, DIM_IN, WIN], f32, kind="ExternalOutput")

    with tile.TileContext(nc) as tc:
        with (
            tc.tile_pool(name="consts", bufs=1) as cpool,
            tc.tile_pool(name="io", bufs=3) as iop,
            tc.tile_pool(name="hbuf", bufs=2) as hpool,
            tc.tile_pool(name="wvbuf", bufs=2) as wvp,
            tc.tile_pool(name="prod", bufs=2) as prodp,
            tc.tile_pool(name="repsmall", bufs=3) as repsb,
            tc.tile_pool(name="efp", bufs=2) as efp,
            tc.tile_pool(name="winp", bufs=2) as winp,
            tc.tile_pool(name="psw", bufs=2, space="PSUM") as psw,
            tc.tile_pool(name="pssmall", bufs=1, space="PSUM") as pssmall,
            tc.tile_pool(name="psseg", bufs=1, space="PSUM") as psseg,
            tc.tile_pool(name="pstr", bufs=1, space="PSUM") as pstr,
        ):
            # ---- constants, loaded once ----
            w1_sb = cpool.tile([EMB, HID], bf16)
            nc.sync.dma_start(out=w1_sb[:], in_=w1[:])
            b1_sb = cpool.tile([128, 2], f32)
            nc.sync.dma_start(
                out=b1_sb[:], in_=b1.rearrange("(c p) o -> p (c o)", p=128)
            )
            w2p_sb = cpool.tile([128, 2, WNUM], bf16)
            nc.sync.dma_start(
                out=w2p_sb[:], in_=w2p.rearrange("(c p) m -> p c m", p=128)
            )
            selab_sb = cpool.tile([128, 32], bf16)
            nc.sync.dma_start(out=selab_sb[:], in_=selab[:])
            selv_sb = cpool.tile([128, 6, 48], bf16)
            nc.sync.dma_start(out=selv_sb[:], in_=selv[:])
            sel48_sb = cpool.tile([128, 2, 48], bf16)
            nc.sync.dma_start(out=sel48_sb[:], in_=sel48[:])
            ident_sb = cpool.tile([64, 64], bf16)
            nc.sync.dma_start(out=ident_sb[:], in_=ident[:])

            act_fn = (
                mybir.ActivationFunctionType.Silu
                if act == "silu"
                else mybir.ActivationFunctionType.Sigmoid
            )

            for g in range(ng):
                win_sb = winp.tile([64, WIN], f32, tag="win")
                for tt in range(GT):
                    t = g * GT + tt
                    tsl = slice(t * T, (t + 1) * T)
                    # ---------------- loads ----------------
                    emb_t = iop.tile([EMB, T], bf16, tag="embt")
                    nc.sync.dma_start(out=emb_t[:], in_=embT[:, tsl])
                    reps_t = iop.tile([128, 6, T], bf16, tag="reps")
                    nc.sync.dma_start(out=reps_t[:], in_=reps[:, :, tsl])
                    fb_t = iop.tile([48, T], bf16, tag="featb")
                    nc.sync.dma_start(out=fb_t[:], in_=featB[:, tsl])
                    oh_t = iop.tile([SUBT, 4, WIN], bf16, tag="oh")
                    nc.sync.dma_start(
                        out=oh_t[:],
                        in_=oh[4 * t : 4 * t + 4].rearrange("s p c -> p s c"),
                    )

                    # ---------------- MLP1: hT = silu(W1.T @ embT + b1) ----
                    hp_ps = psw.tile([128, 2, T], f32, tag="psw")
                    h_sb = []
                    for k in range(2):
                        nc.tensor.matmul(
                            hp_ps[:, k, :],
                            w1_sb[:, k * 128 : (k + 1) * 128],
                            emb_t[:],
                            start=True,
                            stop=True,
                        )
                        h_k = hpool.tile([128, T], bf16, tag=f"h{k}")
                        nc.scalar.activation(
                            h_k[:], hp_ps[:, k, :], act_fn,
                            bias=b1_sb[:, k : k + 1],
                        )
                        h_sb.append(h_k)

                    # ------- MLP2 (paired PSUM banks) + DVE products -------
                    # pair pm covers wT m-blocks {2pm, 2pm+1}; paths:
                    # pm0: ss (* sy0rep), pm1: vv (* crep), pm2: sv (* srep),
                    # pm3: vs (* vy0rep[a], a=0..2; via SBUF since reused 3x)
                    ppair = []
                    for pm in range(3):
                        wp_ps = psw.tile([128, 2, T], f32, tag="psw")
                        for half in range(2):
                            m = 2 * pm + half
                            for k in range(2):
                                nc.tensor.matmul(
                                    wp_ps[:, half, :],
                                    w2p_sb[:, k, m * 128 : (m + 1) * 128],
                                    h_sb[k][:],
                                    start=(k == 0),
                                    stop=(k == 1),
                                )
                        pp = prodp.tile([128, 2, T], bf16, tag=f"pp{pm}")
                        nc.vector.tensor_mul(
                            pp[:],
                            wp_ps[:],
                            reps_t[:, pm : pm + 1, :].to_broadcast([128, 2, T]),
                        )
                        ppair.append(pp)

                    wp3_ps = psw.tile([128, 2, T], f32, tag="psw")
                    for half in range(2):
                        m = 6 + half
                        for k in range(2):
                            nc.tensor.matmul(
                                wp3_ps[:, half, :],
                                w2p_sb[:, k, m * 128 : (m + 1) * 128],
                                h_sb[k][:],
                                start=(k == 0),
                                stop=(k == 1),
                            )
                    wv_sb = wvp.tile([128, 2, T], bf16, tag="wv")
                    nc.scalar.copy(wv_sb[:], wp3_ps[:])
                    p3 = []
                    for a in range(3):
                        pp = prodp.tile([128, 2, T], bf16, tag=f"p3{a}")
                        nc.vector.tensor_mul(
                            pp[:],
                            wv_sb[:],
                            reps_t[:, 3 + a : 4 + a, :].to_broadcast(
                                [128, 2, T]
                            ),
                        )
                        p3.append(pp)

                    # -------- PE column sums (contract over i) --------
                    small_ps = pssmall.tile([128, T], f32, tag="small")
                    small2_ps = pssmall.tile([64, T], f32, tag="small2")
                    os_ps = small_ps[0:16, :]
                    drep_ps = small_ps[64:112, :]
                    ov_ps = small2_ps[0:48, :]

                    sel_a = selab_sb[:, 0:16]
                    sel_b = selab_sb[:, 16:32]
                    nc.tensor.matmul(os_ps, sel_a, ppair[0][:, 0, :], start=True, stop=False)
                    nc.tensor.matmul(os_ps, sel_a, ppair[1][:, 0, :], start=False, stop=False)
                    nc.tensor.matmul(os_ps, sel_b, ppair[0][:, 1, :], start=False, stop=False)
                    nc.tensor.matmul(os_ps, sel_b, ppair[1][:, 1, :], start=False, stop=True)

                    nc.tensor.matmul(drep_ps, sel48_sb[:, 0, :], ppair[2][:, 0, :], start=True, stop=False)
                    nc.tensor.matmul(drep_ps, sel48_sb[:, 1, :], ppair[2][:, 1, :], start=False, stop=True)
                    drep_sb = repsb.tile([48, T], bf16, tag="drepsb")
                    nc.scalar.copy(drep_sb[:], drep_ps)

                    first = True
                    for b in range(2):
                        for a in range(3):
                            nc.tensor.matmul(
                                ov_ps,
                                selv_sb[:, b * 3 + a, :],
                                p3[a][:, b, :],
                                start=first,
                                stop=(b == 1 and a == 2),
                            )
                            first = False

                    # -------- edge features (16-row s part, 48-row v part) --
                    ef_s = efp.tile([16, T], bf16, tag="efs")
                    nc.scalar.copy(ef_s[:], os_ps)
                    ef_v = efp.tile([48, T], bf16, tag="efv")
                    tmpv = efp.tile([48, T], bf16, tag="tmpv")
                    nc.vector.tensor_mul(tmpv[:], drep_sb[:], fb_t[:])
                    nc.vector.tensor_add(ef_v[:], tmpv[:], ov_ps)

                    # -------- scatter: transpose + one-hot matmul --------
                    ef_rs = []
                    for sub in range(4):
                        tr_ps = pstr.tile([128, 64], bf16, tag="tr")
                        esl = slice(sub * 128, (sub + 1) * 128)
                        nc.tensor.transpose(
                            tr_ps[:, 0:16], ef_s[:, esl], ident_sb[0:16, 0:16]
                        )
                        nc.tensor.transpose(
                            tr_ps[:, 16:64], ef_v[:, esl], ident_sb[0:48, 0:48]
                        )
                        ef_r = efp.tile([128, 64], bf16, tag=f"efr{sub}")
                        nc.scalar.copy(ef_r[:], tr_ps[:])
                        ef_rs.append(ef_r)
                    # out[f, n] = sum_e ef_r[e, f] * oh[e, n]
                    seg_ps = psseg.tile([64, WIN], f32, tag="seg")
                    for sub in range(4):
                        nc.tensor.matmul(
                            seg_ps[:],
                            ef_rs[sub][:],
                            oh_t[:, sub, :],
                            start=(sub == 0),
                            stop=(sub == 3),
                        )

                    if tt == 0:
                        nc.scalar.copy(win_sb[:], seg_ps[:])
                    else:
                        nc.vector.tensor_add(win_sb[:], win_sb[:], seg_ps[:])

                nc.sync.dma_start(out=out[g], in_=win_sb[:])

    if split_waits:
        _split_excess_waits(nc)
    return nc


# ---------------------------- host-side prep ------------------------------
def _plan_groups(dst: np.ndarray):
    """Greedy cut into groups of <= GROUP_E edges spanning < WIN node ids."""
    groups = []
    i, n = 0, len(dst)
    while i < n:
        lo = int(dst[i])
        j = min(i + GROUP_E, int(np.searchsorted(dst, lo + WIN, side="left")))
        j = max(j, i + 1)
        groups.append((i, j, lo))
        i = j
    return groups


def _sel_constants():
    r = np.arange(128)
    selab = np.zeros((128, 32), BF16)
    selab[r, r // 16] = 1
    selab[r, 16 + r // 16 + 8] = 1
    selv = np.zeros((128, 6, 48), BF16)
    for b in range(2):
        for a in range(3):
            selv[r, b * 3 + a, (r // 16 + 8 * b) * 3 + a] = 1
    sel48 = np.zeros((128, 2, 48), BF16)
    qq = np.arange(48)
    for b in range(2):
        sel48[:, b, :] = (qq[None, :] // 3 == (r[:, None] // 16 + 8 * b)).astype(
            BF16
        )
    ident = np.eye(64, dtype=BF16)
    return selab, selv, sel48, ident


def _permuted_w2(W2: np.ndarray) -> np.ndarray:
    inv3 = 1.0 / np.sqrt(3.0)
    c_s = np.sqrt(1.0 / (2 * MUL))
    c_v = np.sqrt(3.0 / (2 * MUL))
    z = 1.0 / np.sqrt(E / float(N))
    scale = np.array([c_s * z, c_s * inv3 * z, c_v * inv3 * z, c_v * inv3 * z])
    c = np.arange(WNUM)
    p, rem = c // 256, c % 256
    jj, ii = rem // 16, rem % 16
    src = p * 256 + ii * 16 + jj
    return (W2[:, src] * scale[p][None, :]).astype(F32)


def kernel(
    node_features,
    edge_attr,
    edge_embedding,
    W1,
    b1,
    W2,
    b2,
    edge_src,
    edge_dst,
):
    node_features = np.asarray(node_features, F32)
    edge_attr = np.asarray(edge_attr, F32)
    edge_embedding = np.asarray(edge_embedding, F32)
    W1 = np.asarray(W1, F32)
    b1 = np.asarray(b1, F32)
    W2 = np.asarray(W2, F32)
    b2 = np.asarray(b2, F32)
    edge_src = np.asarray(edge_src)
    edge_dst = np.asarray(edge_dst)

    order = np.argsort(edge_dst, kind="stable")
    per_core = E // NCORES

    core_groups = []
    for c in range(NCORES):
        o = order[c * per_core : (c + 1) * per_core]
        core_groups.append((o, _plan_groups(edge_dst[o])))
    ng = max(len(g) for _, g in core_groups)

    nc = _BUILD_CACHE.get(ng)
    if nc is None:
        nc = _build(ng)
        _BUILD_CACHE[ng] = nc

    nt = ng * GT
    epc = nt * T
    nsub = epc // SUBT
    w2p = _permuted_w2(W2)
    selab, selv, sel48, ident = _sel_constants()
    ridx = np.arange(128) % 16  # partition r holds feature row r%16

    in_maps = []
    los_all = []
    for c in range(NCORES):
        o, groups = core_groups[c]
        perm = np.full(epc, -1, np.int64)
        los = np.zeros(ng, np.int64)
        col = np.zeros(epc, np.int64)
        for g, (i, j, lo) in enumerate(groups):
            k = j - i
            perm[g * GROUP_E : g * GROUP_E + k] = o[i:j]
            los[g] = lo
            col[g * GROUP_E : g * GROUP_E + k] = edge_dst[o[i:j]] - lo
        valid = perm >= 0
        pidx = np.where(valid, perm, 0)

        emb_p = np.where(valid[:, None], edge_embedding[pidx], 0.0).astype(F32)
        x = node_features[edge_src[pidx]]
        y = edge_attr[pidx]
        s = x[:, :MUL]
        v3 = x[:, MUL:].reshape(-1, MUL, 3)
        y0 = y[:, 0:1]
        y1 = y[:, 1:4]
        sy0 = s * y0
        cfeat = (v3 * y1[:, None, :]).sum(-1)
        vy0a = (v3 * y0[:, :, None]).transpose(0, 2, 1).reshape(-1, 48)
        # modulators, already replicated to the 128-partition layout
        # (partition r = jj*16 + i holds feature column i = r%16)
        mods = np.empty((epc, 6, 16), F32)
        mods[:, 0] = sy0
        mods[:, 1] = cfeat
        mods[:, 2] = s
        mods[:, 3] = vy0a[:, 0:16]
        mods[:, 4] = vy0a[:, 16:32]
        mods[:, 5] = vy0a[:, 32:48]
        mods[~valid] = 0.0
        reps_arr = np.ascontiguousarray(
            mods[:, :, ridx].transpose(2, 1, 0).astype(BF16)
        )  # [128, 6, epc]

        feat_b = np.tile(y1, (1, MUL))
        feat_b[~valid] = 0.0

        ohm = np.zeros((epc, WIN), BF16)
        ohm[np.where(valid)[0], col[valid]] = 1

        in_maps.append(
            dict(
                embt=np.ascontiguousarray(emb_p.T.astype(BF16)),
                reps=reps_arr,
                featb=np.ascontiguousarray(feat_b.T.astype(BF16)),
                oh=ohm.reshape(nsub, SUBT, WIN),
                w1=W1.astype(BF16),
                b1=b1.reshape(HID, 1),
                w2p=w2p.astype(BF16),
                selab=selab,
                selv=selv,
                sel48=sel48,
                ident=ident,
            )
        )
        los_all.append(los)

    res = run_bass_kernel_spmd(nc, in_maps, core_ids=list(range(NCORES)))
    global _last_exec_ns, _last_results
    _last_exec_ns = res.exec_time_ns
    _last_results = res

    out = np.zeros((N, DIM_IN), F32)
    for c in range(NCORES):
        blocks = res.results[c]["out"]  # [ng, 64, WIN]
        los = los_all[c]
        ngc = len(core_groups[c][1])
        for g in range(ngc):
            lo = int(los[g])
            hi = min(lo + WIN, N)
            out[lo:hi] += blocks[g].T[: hi - lo]

    if np.any(b2):
        out += _b2_contribution(node_features, edge_attr, b2, edge_src, edge_dst)
    return out


def _b2_contribution(node_features, edge_attr, b2, edge_src, edge_dst):
    inv3 = 1.0 / np.sqrt(3.0)
    c_s = np.sqrt(1.0 / (2 * MUL))
    c_v = np.sqrt(3.0 / (2 * MUL))
    z = 1.0 / np.sqrt(E / float(N))
    w = b2.reshape(4, MUL, MUL)
    x = node_features[edge_src]
    s = x[:, :MUL]
    v = x[:, MUL:].reshape(-1, MUL, 3)
    y0 = edge_attr[:, 0]
    y1 = edge_attr[:, 1:4]
    out_s = c_s * (
        np.einsum("ij,e,ei->ej", w[0], y0, s)
        + inv3 * np.einsum("ij,ea,eia->ej", w[1], y1, v)
    )
    out_v = (c_v * inv3) * (
        np.einsum("ij,ei,ea->eja", w[2], s, y1)
        + np.einsum("ij,eia,e->eja", w[3], v, y0)
    )
    ef = np.concatenate([out_s, out_v.reshape(-1, 3 * MUL)], axis=1)
    out = np.zeros((N, DIM_IN), F32)
    np.add.at(out, edge_dst, ef)
    return (out * z).astype(F32)


# revision 19
# speedup vs baseline: 1.7675x; 1.0456x over previous
"""NequIP-style GNN message passing layer on 8 Trainium2 NeuronCores.

Strategy (edges partitioned across cores per the sharding hint):
  * Host: sort edges by destination, split into 8 contiguous shards, gather
    source-node features per edge, precompute the per-edge tensor-product
    modulators already replicated across the 128-partition layout the device
    needs (sy0, C, s, v*y0 per spherical component), pad edges into
    fixed-size groups whose destinations span < 256 node ids, and build
    per-group one-hot scatter matrices.  All TP path constants and the
    1/sqrt(deg) normalization are folded into a column-permuted copy of W2
    (j-major inside each 16x16 block so the i-contraction is a contiguous
    16-partition column sum).
  * Device (per 512-edge tile; features on partitions, edges on free dim):
      - MLP1/MLP2 as fp32r matmuls -> per-edge TP weights wT [1024, T] in
        paired PSUM banks [128, 2, T]
      - DVE elementwise products w * modulator, mostly straight out of PSUM
      - PE column-sum matmuls with constant 0/1 selection matrices contract
        over the multiplicity index i
      - edge features are PE-transposed to edges-on-partitions and
        scatter-summed via one-hot bf16 matmuls into a [64, 256] window
        accumulated in SBUF, DMA'd out per group (host transposes windows)
  * Host: add the per-window partial sums from all cores into [10000, 64].
"""

import sys

if "/opt/trn_rl_repo" not in sys.path:
    sys.path.insert(0, "/opt/trn_rl_repo")

import numpy as np
import ml_dtypes

import concourse.bass as bass
import concourse.mybir as mybir
import concourse.tile as tile
from concourse.tile import ScopedClock
from concourse.bass_utils import run_bass_kernel_spmd
from concourse import bass_utils as _bass_utils

if getattr(_bass_utils, "_ldw_patch", None) is not True:
    import os as _os

    if _os.environ.get("KERNEL_LDW_OPT") == "1":
        _orig_run_command = _bass_utils.run_command

        def _run_command_ldw(argv, **kw):
            argv = [
                "--enable-ldw-opt=true" if a == "--enable-ldw-opt=false" else a
                for a in argv
            ]
            return _orig_run_command(argv, **kw)

        _bass_utils.run_command = _run_command_ldw
    _bass_utils._ldw_patch = True

# ---------------- problem constants (hardcoded per contract) ----------------
N = 10000
E = 160000
MUL = 16
DIM_IN = 64
DIM_SH = 4
EMB = 18
HID = 256
WNUM = 1024

NCORES = 8
T = 512          # edges per tile
SUBT = 128       # edges per subtile (partition dim for scatter)
WIN = 256        # node window per group
GT = 4           # tiles per group
GROUP_E = GT * T

BF16 = ml_dtypes.bfloat16
F32 = np.float32

_BUILD_CACHE = {}
_last_exec_ns = None
_last_results = None


# --------------------------------------------------------------------------
# walrus's setupSyncWait in this container rejects instructions with more
# than one sem wait; Tile can emit several.  Two fixes: split the tail
# drain's waits across consecutive drains, and a post-pass that moves excess
# waits from any instruction onto same-engine NOPs placed right before it
# (identical semantics: waits AND together, the engine stalls either way).
# --------------------------------------------------------------------------
def _patched_drain_and_barrier(self, tick_clock, wait_clock):
    drain_inst = self.nc.sync.drain()
    wait_clock.add_sem_waits(
        drain_inst.ins, ScopedClock({None: tick_clock.global_clock})
    )
    si = drain_inst.ins.sync_info
    if si is not None and si.on_wait and len(si.on_wait) > 1:
        waits = list(si.on_wait)
        drain_inst.ins.sync_info = mybir.SyncInfo(
            on_wait=waits[:1], on_update=list(si.on_update or [])
        )
        for w in waits[1:]:
            extra = self.nc.sync.drain()
            extra.ins.sync_info = mybir.SyncInfo(on_wait=[w], on_update=[])
    self.nc.all_engine_barrier()
    assert self.sems is not None
    popped = self.nc._tile_sem_poison_stack.pop()
    assert popped is self._sem_poison
    self.nc.clear_and_free_semaphores(list(self.sems.allocated().values()))
    self.nc.all_engine_barrier()


if getattr(tile.TileContext, "_drain_patch", None) is not True:
    tile.TileContext._drain_and_barrier = _patched_drain_and_barrier
    tile.TileContext._drain_patch = True


def _split_excess_waits(nc: bass.Bass, maxw: int = 1) -> None:
    for f in nc.m.functions:
        for bb in f.blocks:
            rewritten = []
            changed = False
            for inst in bb.instructions:
                si = inst.sync_info
                if si is not None and si.on_wait and len(si.on_wait) > maxw:
                    waits = list(si.on_wait)
                    extra, keep = waits[:-maxw], waits[-maxw:]
                    for i, w in enumerate(extra):
                        nop = mybir.InstNoOp(
                            name=f"{inst.name}-ws{i}",
                            engine=inst.engine,
                            ins=[],
                            outs=[],
                        )
                        nop.sync_info = mybir.SyncInfo(on_wait=[w], on_update=[])
                        rewritten.append(nop)
                    inst.sync_info = mybir.SyncInfo(
                        on_wait=keep, on_update=list(si.on_update or [])
                    )
                    changed = True
                rewritten.append(inst)
            if changed:
                bb.instructions = rewritten


# ---------------------------- device program ------------------------------
def _build(ng: int, split_waits: bool = True, act: str = "silu") -> bass.Bass:
    """Build the per-core Bass program for ng groups (= ng*GT tiles)."""
    f32 = mybir.dt.float32
    f32r = mybir.dt.float32r
    bf16 = mybir.dt.bfloat16

    nt = ng * GT
    epc = nt * T
    nsub = epc // SUBT

    nc = bass.Bass(trn_type="TRN2")

    embT = nc.dram_tensor("embt", [EMB, epc], bf16, kind="ExternalInput")
    reps = nc.dram_tensor("reps", [128, 6, epc], bf16, kind="ExternalInput")
    featB = nc.dram_tensor("featb", [48, epc], bf16, kind="ExternalInput")
    oh = nc.dram_tensor("oh", [nsub, SUBT, WIN], bf16, kind="ExternalInput")
    w1 = nc.dram_tensor("w1", [EMB, HID], bf16, kind="ExternalInput")
    b1 = nc.dram_tensor("b1", [HID, 1], f32, kind="ExternalInput")
    w2p = nc.dram_tensor("w2p", [HID, WNUM], bf16, kind="ExternalInput")
    selab = nc.dram_tensor("selab", [128, 32], bf16, kind="ExternalInput")
    selv = nc.dram_tensor("selv", [128, 6, 48], bf16, kind="ExternalInput")
    sel48 = nc.dram_tensor("sel48", [128, 2, 48], bf16, kind="ExternalInput")
    ident = nc.dram_tensor("ident", [80, 80], bf16, kind="ExternalInput")
    out = nc.dram_tensor("out", [ng, DIM_IN, WIN], f32, kind="ExternalOutput")

    with tile.TileContext(nc) as tc:
        with (
            tc.tile_pool(name="consts", bufs=1) as cpool,
            tc.tile_pool(name="io", bufs=3) as iop,
            tc.tile_pool(name="hbuf", bufs=2) as hpool,
            tc.tile_pool(name="wvbuf", bufs=2) as wvp,
            tc.tile_pool(name="prod", bufs=2) as prodp,
            tc.tile_pool(name="repsmall", bufs=3) as repsb,
            tc.tile_pool(name="efp", bufs=2) as efp,
            tc.tile_pool(name="psw", bufs=2, space="PSUM") as psw,
            tc.tile_pool(name="pssmall", bufs=1, space="PSUM") as pssmall,
            tc.tile_pool(name="psseg", bufs=1, space="PSUM") as psseg,
            tc.tile_pool(name="pstr", bufs=1, space="PSUM") as pstr,
        ):
            # ---- constants, loaded once ----
            w1_sb = cpool.tile([EMB, HID], bf16)
            nc.sync.dma_start(out=w1_sb[:], in_=w1[:])
            b1_sb = cpool.tile([128, 2], f32)
            nc.sync.dma_start(
                out=b1_sb[:], in_=b1.rearrange("(c p) o -> p (c o)", p=128)
            )
            w2p_sb = cpool.tile([128, 2, WNUM], bf16)
            nc.sync.dma_start(
                out=w2p_sb[:], in_=w2p.rearrange("(c p) m -> p c m", p=128)
            )
            selab_sb = cpool.tile([128, 32], bf16)
            nc.sync.dma_start(out=selab_sb[:], in_=selab[:])
            selv_sb = cpool.tile([128, 6, 48], bf16)
            nc.sync.dma_start(out=selv_sb[:], in_=selv[:])
            sel48_sb = cpool.tile([128, 2, 48], bf16)
            nc.sync.dma_start(out=sel48_sb[:], in_=sel48[:])
            ident_sb = cpool.tile([80, 80], bf16)
            nc.sync.dma_start(out=ident_sb[:], in_=ident[:])

            act_fn = (
                mybir.ActivationFunctionType.Silu
                if act == "silu"
                else mybir.ActivationFunctionType.Sigmoid
            )

            for g in range(ng):
                seg_ps = psseg.tile([64, WIN], f32, tag="seg")
                for tt in range(GT):
                    t = g * GT + tt
                    tsl = slice(t * T, (t + 1) * T)
                    # ---------------- loads ----------------
                    emb_t = iop.tile([EMB, T], bf16, tag="embt")
                    nc.sync.dma_start(out=emb_t[:], in_=embT[:, tsl])
                    reps_t = iop.tile([128, 6, T], bf16, tag="reps")
                    nc.sync.dma_start(out=reps_t[:], in_=reps[:, :, tsl])
                    fb_t = iop.tile([48, T], bf16, tag="featb")
                    nc.sync.dma_start(out=fb_t[:], in_=featB[:, tsl])
                    oh_t = iop.tile([SUBT, 4, WIN], bf16, tag="oh")
                    nc.sync.dma_start(
                        out=oh_t[:],
                        in_=oh[4 * t : 4 * t + 4].rearrange("s p c -> p s c"),
                    )

                    # ---------------- MLP1: hT = silu(W1.T @ embT + b1) ----
                    hp_ps = psw.tile([128, 2, T], f32, tag="psw")
                    h_sb = []
                    for k in range(2):
                        nc.tensor.matmul(
                            hp_ps[:, k, :],
                            w1_sb[:, k * 128 : (k + 1) * 128],
                            emb_t[:],
                            start=True,
                            stop=True,
                        )
                        h_k = hpool.tile([128, T], bf16, tag=f"h{k}")
                        nc.scalar.activation(
                            h_k[:], hp_ps[:, k, :], act_fn,
                            bias=b1_sb[:, k : k + 1],
                        )
                        h_sb.append(h_k)

                    # ------- MLP2 (paired PSUM banks) + DVE products -------
                    # pair pm covers wT m-blocks {2pm, 2pm+1}; paths:
                    # pm0: ss (* sy0rep), pm1: vv (* crep), pm2: sv (* srep),
                    # pm3: vs (* vy0rep[a], a=0..2; via SBUF since reused 3x)
                    ppair = []
                    for pm in range(3):
                        wp_ps = psw.tile([128, 2, T], f32, tag="psw")
                        for half in range(2):
                            m = 2 * pm + half
                            for k in range(2):
                                nc.tensor.matmul(
                                    wp_ps[:, half, :],
                                    w2p_sb[:, k, m * 128 : (m + 1) * 128],
                                    h_sb[k][:],
                                    start=(k == 0),
                                    stop=(k == 1),
                                )
                        pp = prodp.tile([128, 2, T], bf16, tag=f"pp{pm}")
                        nc.vector.tensor_mul(
                            pp[:],
                            wp_ps[:],
                            reps_t[:, pm : pm + 1, :].to_broadcast([128, 2, T]),
                        )
                        ppair.append(pp)

                    wp3_ps = psw.tile([128, 2, T], f32, tag="psw")
                    for half in range(2):
                        m = 6 + half
                        for k in range(2):
                            nc.tensor.matmul(
                                wp3_ps[:, half, :],
                                w2p_sb[:, k, m * 128 : (m + 1) * 128],
                                h_sb[k][:],
                                start=(k == 0),
                                stop=(k == 1),
                            )
                    wv_sb = wvp.tile([128, 2, T], bf16, tag="wv")
                    nc.scalar.copy(wv_sb[:], wp3_ps[:])
                    p3 = []
                    for a in range(3):
                        pp = prodp.tile([128, 2, T], bf16, tag=f"p3{a}")
                        nc.vector.tensor_mul(
                            pp[:],
                            wv_sb[:],
                            reps_t[:, 3 + a : 4 + a, :].to_broadcast(
                                [128, 2, T]
                            ),
                        )
                        p3.append(pp)

                    # -------- PE column sums (contract over i) --------
                    small_ps = pssmall.tile([128, T], f32, tag="small")
                    small2_ps = pssmall.tile([64, T], f32, tag="small2")
                    os_ps = small_ps[0:16, :]
                    drep_ps = small_ps[64:112, :]
                    ov_ps = small2_ps[0:48, :]

                    sel_a = selab_sb[:, 0:16]
                    sel_b = selab_sb[:, 16:32]
                    nc.tensor.matmul(os_ps, sel_a, ppair[0][:, 0, :], start=True, stop=False)
                    nc.tensor.matmul(os_ps, sel_a, ppair[1][:, 0, :], start=False, stop=False)
                    nc.tensor.matmul(os_ps, sel_b, ppair[0][:, 1, :], start=False, stop=False)
                    nc.tensor.matmul(os_ps, sel_b, ppair[1][:, 1, :], start=False, stop=True)

                    nc.tensor.matmul(drep_ps, sel48_sb[:, 0, :], ppair[2][:, 0, :], start=True, stop=False)
                    nc.tensor.matmul(drep_ps, sel48_sb[:, 1, :], ppair[2][:, 1, :], start=False, stop=True)
                    drep_sb = repsb.tile([48, T], bf16, tag="drepsb")
                    nc.scalar.copy(drep_sb[:], drep_ps)

                    first = True
                    for b in range(2):
                        for a in range(3):
                            nc.tensor.matmul(
                                ov_ps,
                                selv_sb[:, b * 3 + a, :],
                                p3[a][:, b, :],
                                start=first,
                                stop=(b == 1 and a == 2),
                            )
                            first = False

                    # ---- edge features: one 80-row tile (v 0:48, s 64:80;
                    #      rows 48:64 zeroed so the transpose stays finite) ----
                    ef80 = efp.tile([80, T], bf16, tag="ef80")
                    nc.gpsimd.memset(ef80[32:64, :], 0.0)  # rows 32:48 rewritten by ef_v below
                    nc.scalar.copy(ef80[64:80, :], os_ps)
                    tmpv = efp.tile([48, T], bf16, tag="tmpv")
                    nc.vector.tensor_mul(tmpv[:], drep_sb[:], fb_t[:])
                    nc.vector.tensor_add(ef80[0:48, :], tmpv[:], ov_ps)

                    # -------- scatter: transpose + one-hot matmul --------
                    for sub in range(4):
                        tr_ps = pstr.tile([128, 80], bf16, tag="tr")
                        esl = slice(sub * 128, (sub + 1) * 128)
                        nc.tensor.transpose(
                            tr_ps[:], ef80[:, esl], ident_sb[:]
                        )
                        ef_r = efp.tile([128, 64], bf16, tag=f"efr{sub}")
                        nc.scalar.copy(ef_r[:, 16:64], tr_ps[:, 0:48])
                        nc.scalar.copy(ef_r[:, 0:16], tr_ps[:, 64:80])
                        # out[f, n] = sum_e ef_r[e, f] * oh[e, n]
                        nc.tensor.matmul(
                            seg_ps[:],
                            ef_r[:],
                            oh_t[:, sub, :],
                            start=(tt == 0 and sub == 0),
                            stop=(tt == GT - 1 and sub == 3),
                        )

                win_sb = efp.tile([64, WIN], f32, tag="winflush")
                nc.scalar.copy(win_sb[:], seg_ps[:])
                nc.sync.dma_start(out=out[g], in_=win_sb[:])

    if split_waits:
        _split_excess_waits(nc)
    return nc


# ---------------------------- host-side prep ------------------------------
def _plan_groups(dst: np.ndarray):
    """Greedy cut into groups of <= GROUP_E edges spanning < WIN node ids."""
    groups = []
    i, n = 0, len(dst)
    while i < n:
        lo = int(dst[i])
        j = min(i + GROUP_E, int(np.searchsorted(dst, lo + WIN, side="left")))
        j = max(j, i + 1)
        groups.append((i, j, lo))
        i = j
    return groups


def _sel_constants():
    r = np.arange(128)
    selab = np.zeros((128, 32), BF16)
    selab[r, r // 16] = 1
    selab[r, 16 + r // 16 + 8] = 1
    selv = np.zeros((128, 6, 48), BF16)
    for b in range(2):
        for a in range(3):
            selv[r, b * 3 + a, (r // 16 + 8 * b) * 3 + a] = 1
    sel48 = np.zeros((128, 2, 48), BF16)
    qq = np.arange(48)
    for b in range(2):
        sel48[:, b, :] = (qq[None, :] // 3 == (r[:, None] // 16 + 8 * b)).astype(
            BF16
        )
    ident = np.eye(80, dtype=BF16)
    return selab, selv, sel48, ident


def _permuted_w2(W2: np.ndarray) -> np.ndarray:
    inv3 = 1.0 / np.sqrt(3.0)
    c_s = np.sqrt(1.0 / (2 * MUL))
    c_v = np.sqrt(3.0 / (2 * MUL))
    z = 1.0 / np.sqrt(E / float(N))
    scale = np.array([c_s * z, c_s * inv3 * z, c_v * inv3 * z, c_v * inv3 * z])
    c = np.arange(WNUM)
    p, rem = c // 256, c % 256
    jj, ii = rem // 16, rem % 16
    src = p * 256 + ii * 16 + jj
    return (W2[:, src] * scale[p][None, :]).astype(F32)


def kernel(
    node_features,
    edge_attr,
    edge_embedding,
    W1,
    b1,
    W2,
    b2,
    edge_src,
    edge_dst,
):
    node_features = np.asarray(node_features, F32)
    edge_attr = np.asarray(edge_attr, F32)
    edge_embedding = np.asarray(edge_embedding, F32)
    W1 = np.asarray(W1, F32)
    b1 = np.asarray(b1, F32)
    W2 = np.asarray(W2, F32)
    b2 = np.asarray(b2, F32)
    edge_src = np.asarray(edge_src)
    edge_dst = np.asarray(edge_dst)

    order = np.argsort(edge_dst, kind="stable")
    per_core = E // NCORES

    core_groups = []
    for c in range(NCORES):
        o = order[c * per_core : (c + 1) * per_core]
        core_groups.append((o, _plan_groups(edge_dst[o])))
    ng = max(len(g) for _, g in core_groups)

    nc = _BUILD_CACHE.get(ng)
    if nc is None:
        nc = _build(ng)
        _BUILD_CACHE[ng] = nc

    nt = ng * GT
    epc = nt * T
    nsub = epc // SUBT
    w2p = _permuted_w2(W2)
    selab, selv, sel48, ident = _sel_constants()
    ridx = np.arange(128) % 16  # partition r holds feature row r%16

    in_maps = []
    los_all = []
    for c in range(NCORES):
        o, groups = core_groups[c]
        perm = np.full(epc, -1, np.int64)
        los = np.zeros(ng, np.int64)
        col = np.zeros(epc, np.int64)
        for g, (i, j, lo) in enumerate(groups):
            k = j - i
            perm[g * GROUP_E : g * GROUP_E + k] = o[i:j]
            los[g] = lo
            col[g * GROUP_E : g * GROUP_E + k] = edge_dst[o[i:j]] - lo
        valid = perm >= 0
        pidx = np.where(valid, perm, 0)

        emb_p = np.where(valid[:, None], edge_embedding[pidx], 0.0).astype(F32)
        x = node_features[edge_src[pidx]]
        y = edge_attr[pidx]
        s = x[:, :MUL]
        v3 = x[:, MUL:].reshape(-1, MUL, 3)
        y0 = y[:, 0:1]
        y1 = y[:, 1:4]
        sy0 = s * y0
        cfeat = (v3 * y1[:, None, :]).sum(-1)
        vy0a = (v3 * y0[:, :, None]).transpose(0, 2, 1).reshape(-1, 48)
        # modulators, already replicated to the 128-partition layout
        # (partition r = jj*16 + i holds feature column i = r%16)
        mods = np.empty((epc, 6, 16), F32)
        mods[:, 0] = sy0
        mods[:, 1] = cfeat
        mods[:, 2] = s
        mods[:, 3] = vy0a[:, 0:16]
        mods[:, 4] = vy0a[:, 16:32]
        mods[:, 5] = vy0a[:, 32:48]
        mods[~valid] = 0.0
        reps_arr = np.ascontiguousarray(
            mods[:, :, ridx].transpose(2, 1, 0).astype(BF16)
        )  # [128, 6, epc]

        feat_b = np.tile(y1, (1, MUL))
        feat_b[~valid] = 0.0

        ohm = np.zeros((epc, WIN), BF16)
        ohm[np.where(valid)[0], col[valid]] = 1

        in_maps.append(
            dict(
                embt=np.ascontiguousarray(emb_p.T.astype(BF16)),
                reps=reps_arr,
                featb=np.ascontiguousarray(feat_b.T.astype(BF16)),
                oh=ohm.reshape(nsub, SUBT, WIN),
                w1=W1.astype(BF16),
                b1=b1.reshape(HID, 1),
                w2p=w2p.astype(BF16),
                selab=selab,
                selv=selv,
                sel48=sel48,
                ident=ident,
            )
        )
        los_all.append(los)

    res = run_bass_kernel_spmd(nc, in_maps, core_ids=list(range(NCORES)))
    global _last_exec_ns, _last_results
    _last_exec_ns = res.exec_time_ns
    _last_results = res

    out = np.zeros((N, DIM_IN), F32)
    for c in range(NCORES):
        blocks = res.results[c]["out"]  # [ng, 64, WIN]
        los = los_all[c]
        ngc = len(core_groups[c][1])
        for g in range(ngc):
            lo = int(los[g])
            hi = min(lo + WIN, N)
            out[lo:hi] += blocks[g].T[: hi - lo]

    if np.any(b2):
        out += _b2_contribution(node_features, edge_attr, b2, edge_src, edge_dst)
    return out


def _b2_contribution(node_features, edge_attr, b2, edge_src, edge_dst):
    inv3 = 1.0 / np.sqrt(3.0)
    c_s = np.sqrt(1.0 / (2 * MUL))
    c_v = np.sqrt(3.0 / (2 * MUL))
    z = 1.0 / np.sqrt(E / float(N))
    w = b2.reshape(4, MUL, MUL)
    x = node_features[edge_src]
    s = x[:, :MUL]
    v = x[:, MUL:].reshape(-1, MUL, 3)
    y0 = edge_attr[:, 0]
    y1 = edge_attr[:, 1:4]
    out_s = c_s * (
        np.einsum("ij,e,ei->ej", w[0], y0, s)
        + inv3 * np.einsum("ij,ea,eia->ej", w[1], y1, v)
    )
    out_v = (c_v * inv3) * (
        np.einsum("ij,ei,ea->eja", w[2], s, y1)
        + np.einsum("ij,eia,e->eja", w[3], v, y0)
    )
    ef = np.concatenate([out_s, out_v.reshape(-1, 3 * MUL)], axis=1)
    out = np.zeros((N, DIM_IN), F32)
    np.add.at(out, edge_dst, ef)
    return (out * z).astype(F32)
